# revision 55
# baseline (speedup 1.0000x reference)
"""Trainium2 Bass kernel for nn_AttentionBlock_68624987455817.

Pre-LN causal self-attention block + MLP (B=8, L=1024, E=768, H=12, D=64).

Sharding: data-parallel over batch B=8 across the 8 NeuronCores (one batch
element per core, weights replicated, no collectives). Each core runs the
full block on its [1024, 768] slice.

Optimized from the first working kernel (391us) to ~335us. The changes that
mattered, in order of impact:
  - all transposes on the PE (identity matmul) — the DMA-transpose path left
    the PE idle ~100us across the LN1 prologue and the LN2 mid-kernel valley.
  - per-tile LN stats (bn_stats/bn_aggr, stats emitted 3 tiles ahead) so each
    token tile's stats -> apply -> transpose -> matmul chain pipelines
    instead of barriering on all 8 tiles; LN1 fused with the V matmuls.
  - x1 (attention residual) stays in SBUF; the DRAM store/reload roundtrip
    and the separate LN2 pass are gone. proj -> LN2 -> transpose is
    software-pipelined by one tile so the PE never waits on the DVE chain.
  - ph3 software pipeline (see below): the exp/mask/copy chains of pair c
    complete under pair c+1's qk matmul stream; PV(q1) accumulation is split
    around the second S^T batch so it never waits on fresh exps.  Without
    this the recurring sub-us PE stalls also re-trigger the HAM clock gate
    (PE drops to 1.2 GHz) — the stall cost roughly doubles itself.
  - matmul operand dtype defaults to bf16 (1 cyc/col + FWL weight loads);
    rel err ~3.2e-3 vs the 2e-2 gate.  Counterintuitively bf16 alone was
    NOT faster than f32r (f32r also streams ~1 col/cycle warm); the wins
    came from the scheduling changes above.
  - whole wfc prefetched during ph4, wout passes A+B during ph5, wv/wproj
    during ph0; x tiles 0-1 DMA'd before any setup; ACT function tables
    (rsqrt/exp) preloaded off the critical path (a table switch is 1.3us).
  - engine placement tuned: psum->SBUF copies split across Vector/Scalar,
    masks+normalize on Vector, partition-broadcast on GpSimd.  GpSimd is
    ONLY safe for ops whose consumers are far away (its strict FIFO +
    per-op library reload otherwise serialize the whole pipeline — moving
    masks or LN applies there cost 2x overall).

Per-core dataflow (activations feature-major through the matmuls):
  ph0+2 per token tile t: LN1 stats (bn_stats) -> z tile -> PE transpose
        into z1T -> V matmuls (ones column per head makes P@V emit softmax
        row-sums at psum row 64).
  ph3   per head pair (2c, 2c+1), cycle: S^T(kt0-3) -> qk(c+1) -> PV(q0) ->
        PV(q1) over kt0-3 -> S^T(kt4-7) -> PV(q1) tail -> normalize.
        S^T = k_h^T q_h in [P, L] psums (seg-major emission so the two
        parities' disjoint PE row groups pack); exp -> P^T (masked);
        [O^T; sums] = Vaug^T P^T; normalize via fast reciprocal + gpsimd
        partition broadcast.
  ph4   per tile: x1 = O @ wproj + x (SBUF-resident); LN2 stats -> z2 ->
        PE transpose into z2T (pipelined one tile behind the proj matmuls).
  ph5   hT = selu(wfc^T @ z2T)  (wfc pre-scaled by selu lambda)
  ph6   out = h @ wout + x1     (two column passes; wout prefetched in ph5)

LN scales fold into the following weight matrices host-side; LN biases and
all linear biases fold into per-feature biases only materialized on-chip
when nonzero (all zero for this problem's inputs).
"""
import os
import sys

sys.path.insert(0, "/opt/trn_rl_repo")

import numpy as np
import ml_dtypes

import concourse.bass as bass
from concourse import bacc
import concourse.mybir as mybir
from concourse.tile import TileContext
from concourse import bass_utils
from concourse.masks import make_identity

F32 = mybir.dt.float32
F32R = mybir.dt.float32r
BF16 = mybir.dt.bfloat16
AF = mybir.ActivationFunctionType
OP = mybir.AluOpType
AX = mybir.AxisListType

P = 128
L = 1024
E = 768
H = 12
D = 64
DA = D + 1           # V columns + ones column (row-sum trick)
EC = E // P          # 6 feature chunks
LT = L // P          # 8 token tiles
QC = L // 512        # 2 query chunks
KC2 = 4 * E // P     # 24 chunks of the MLP hidden dim
NCORES = 8

SELU_LAMBDA = 1.0507009873554805
SELU_ALPHA = 1.6732632423543772
SELU_LA = SELU_LAMBDA * SELU_ALPHA
LN_EPS = 1e-6

_last_results = None
_build_cache = {}


def _build(gates, mm_dt_name):
    MDT = {"f32r": F32R, "bf16": BF16}[mm_dt_name]
    PSDT = MDT  # transpose output dtype must match its input dtype

    nc = bacc.Bacc("TRN2", target_bir_lowering=False)

    x_d = nc.dram_tensor("x", [L, E], F32, kind="ExternalInput")
    wqk_d = nc.dram_tensor("wqk", [E, 2 * E], MDT, kind="ExternalInput")
    wv_d = nc.dram_tensor("wv", [E, E], MDT, kind="ExternalInput")
    wproj_d = nc.dram_tensor("wproj", [E, E], MDT, kind="ExternalInput")
    wfc_d = nc.dram_tensor("wfc", [E, 4 * E], MDT, kind="ExternalInput")
    wout_d = nc.dram_tensor("wout", [4 * E, E], MDT, kind="ExternalInput")
    out_d = nc.dram_tensor("out", [L, E], F32, kind="ExternalOutput")

    bqk_d = bv_d = bproj_d = bfce_d = bfcl_d = bout_d = None
    if gates["bqk"]:
        bqk_d = nc.dram_tensor("bqk", [2 * E], F32, kind="ExternalInput")
    if gates["bv"]:
        bv_d = nc.dram_tensor("bv", [E], F32, kind="ExternalInput")
    if gates["bproj"]:
        bproj_d = nc.dram_tensor("bproj", [E], F32, kind="ExternalInput")
    if gates["bfc"]:
        bfce_d = nc.dram_tensor("bfce", [4 * E], F32, kind="ExternalInput")
        bfcl_d = nc.dram_tensor("bfcl", [4 * E], F32, kind="ExternalInput")
    if gates["bout"]:
        bout_d = nc.dram_tensor("bout", [E], F32, kind="ExternalInput")

    xv = x_d.rearrange("(t p) e -> p t e", p=P)            # [128, 8, 768]
    wqkv = wqk_d.rearrange("(c p) m -> p c m", p=P)        # [128, 6, 1536]
    wvv = wv_d.rearrange("(c p) m -> p c m", p=P)          # [128, 6, 768]
    wprojv = wproj_d.rearrange("(c p) m -> p c m", p=P)    # [128, 6, 768]
    wfcv = wfc_d.rearrange("(c p) m -> p c m", p=P)        # [128, 6, 3072]
    woutv = wout_d.rearrange("(c p) m -> p c m", p=P)      # [128, 24, 768]
    outv = out_d.rearrange("(t p) e -> p t e", p=P)

    with TileContext(nc) as tc:
        with tc.tile_pool(name="pers", bufs=1) as pers:
            xall = pers.tile([P, LT, E], F32)    # x tiles, live ph0 -> ph4
            x1all = pers.tile([P, LT, E], F32)   # x1 tiles, live ph4 -> ph6
            # x tiles 0-1 gate the whole LN1 chain: issue their DMAs before
            # any setup so the transfer overlaps the constant initialization.
            for t in range(2):
                nc.sync.dma_start(xall[:, t, :], xv[:, t, :])
            # preload the rsqrt ACT table while the DMAs run so the first LN
            # chain doesn't eat the 1.3us table load; the exp table is
            # preloaded at ph3 entry (ACT holds one table at a time).
            tabs = pers.tile([P, 1], F32)
            nc.vector.memset(tabs[:], 0.5)
            nc.scalar.activation(tabs[:], tabs[:], AF.Abs_reciprocal_sqrt)

            # mask_tri[p, f] = 1.0 if f >= p else 0.0 (keep where k <= q).
            # Built in f32 (f32r memset/affine_select fail walrus codegen).
            mask_f32 = pers.tile([P, P], F32)
            nc.gpsimd.memset(mask_f32[:], 0.0)
            nc.gpsimd.affine_select(
                out=mask_f32[:], in_=mask_f32[:],
                compare_op=OP.is_ge, fill=1.0, base=-1,
                pattern=[[-1, P]], channel_multiplier=1,
            )
            if MDT == F32R:
                mask_tri = mask_f32[:].bitcast(F32R)
            else:
                mask_b = pers.tile([P, P], BF16)
                nc.vector.tensor_copy(mask_b[:], mask_f32[:])
                mask_tri = mask_b[:]
            ones_f32 = pers.tile([P, LT * H], F32)
            nc.vector.memset(ones_f32[:], 1.0)
            eps_b = pers.tile([P, 1], F32)
            nc.vector.memset(eps_b[:], LN_EPS)
            lnla_b = pers.tile([P, 1], F32)
            nc.vector.memset(lnla_b[:], float(np.log(SELU_LA)))

            ident = pers.tile([P, P], F32)
            make_identity(nc, ident)
            ident_m = pers.tile([P, P], MDT)
            nc.vector.tensor_copy(ident_m[:], ident[:])

            bqk_sb = bv_sb = bproj_sb = bfce_sb = bfcl_sb = bout_sb = None
            if gates["bqk"]:
                bqk_sb = pers.tile([P, 2 * EC], F32)
                nc.sync.dma_start(bqk_sb[:], bqk_d.rearrange("(c p) -> p c", p=P))
            if gates["bv"]:
                bv_sb = pers.tile([P, E], F32)
                nc.sync.dma_start(bv_sb[:], bv_d.to_broadcast((P, E)))
            if gates["bproj"]:
                bproj_sb = pers.tile([P, E], F32)
                nc.sync.dma_start(bproj_sb[:], bproj_d.to_broadcast((P, E)))
            if gates["bfc"]:
                bfce_sb = pers.tile([P, KC2], F32)
                nc.sync.dma_start(bfce_sb[:], bfce_d.rearrange("(c p) -> p c", p=P))
                bfcl_sb = pers.tile([P, KC2], F32)
                nc.sync.dma_start(bfcl_sb[:], bfcl_d.rearrange("(c p) -> p c", p=P))
            if gates["bout"]:
                bout_sb = pers.tile([P, E], F32)
                nc.sync.dma_start(bout_sb[:], bout_d.to_broadcast((P, E)))

            def transpose_block(dstT, src_tile, t, pspool):
                """dstT[:, c, t*P:(t+1)*P] = src_tile[:, c*P:(c+1)*P].T"""
                for c in range(EC):
                    pt = pspool.tile([P, P], PSDT, tag="tr")
                    nc.tensor.transpose(pt[:], src_tile[:, c * P:(c + 1) * P],
                                        ident_m[:])
                    nc.scalar.copy(out=dstT[:, c, t * P:(t + 1) * P],
                                   in_=pt[:])

            def ln_tile(src, stpool, tag):
                """bn_stats mean/var over the free axis + rsqrt(var+eps)."""
                bnst = stpool.tile([P, 2, 6], F32, tag=f"bn{tag}", name="bnst")
                xg = src.rearrange("p (n f) -> p n f", f=384)
                for g in range(2):
                    nc.vector.bn_stats(bnst[:, g, :], xg[:, g, :])
                mv = stpool.tile([P, 2], F32, tag=f"mv{tag}", name="mv")
                nc.vector.bn_aggr(mv[:], bnst[:])
                rt = stpool.tile([P, 1], F32, tag=f"rt{tag}", name="rt")
                nc.scalar.activation(rt[:], mv[:, 1:2], AF.Abs_reciprocal_sqrt,
                                     bias=eps_b[:])
                return mv, rt

            with tc.tile_pool(name="fm", bufs=1) as fmp:
                z1T = fmp.tile([P, EC, L], MDT, tag="fm", name="z1T")
                with tc.tile_pool(name="wfcp", bufs=1) as wfcp:
                    wfc_sb = wfcp.tile([P, EC, 4 * E], MDT)
                    # raw pre-selu outputs of the fc groups computed early
                    # inside ph4 (selu applied in ph5 to avoid rsqrt/exp
                    # ACT-table thrash)
                    raw6 = wfcp.tile([P, 6, 512], BF16)
                    with (
                        tc.tile_pool(name="otp", bufs=1) as otp,
                        tc.tile_pool(name="vp", bufs=1) as vpool,
                        tc.tile_pool(name="wpp", bufs=1) as wpp,
                    ):
                        OT = otp.tile([P, EC, L], MDT)
                        v_aug = vpool.tile([P, LT, H, DA], MDT)
                        wproj_sb = wpp.tile([P, EC, E], MDT)

                        # ---- ph0+ph2 fused: per-tile LN1 -> z1T -> V ----
                        with (
                            tc.tile_pool(name="wvp", bufs=1) as wvp,
                            tc.tile_pool(name="zp", bufs=2) as zp,
                            tc.tile_pool(name="stp", bufs=4) as stp,
                            tc.tile_pool(name="ps0", bufs=4, space="PSUM") as ps0,
                            tc.tile_pool(name="ps2", bufs=4, space="PSUM") as ps2,
                        ):
                            wv_sb = wvp.tile([P, EC, E], MDT)
                            nc.vector.tensor_copy(
                                v_aug[:, :, :, D:DA],
                                ones_f32[:].rearrange("p (t h o) -> p t h o",
                                                      h=H, o=1))
                            # x tiles 0-1 already in flight (issued at pers
                            # setup); wv next (needed by V at ~7us), then the
                            # rest of x, then wproj.
                            nc.sync.dma_start(wv_sb[:], wvv[:])
                            for t in range(2, LT):
                                nc.sync.dma_start(xall[:, t, :], xv[:, t, :])
                            nc.sync.dma_start(wproj_sb[:], wprojv[:])
                            # LN1 stats run 3 tiles ahead so the ACT rsqrt is
                            # never queued behind transpose copies.
                            stats1 = {}
                            for t in range(3):
                                stats1[t] = ln_tile(xall[:, t, :], stp, "1")
                            for t in range(LT):
                                if t + 3 < LT:
                                    stats1[t + 3] = ln_tile(
                                        xall[:, t + 3, :], stp, "1")
                                mv, rt = stats1.pop(t)
                                zt = zp.tile([P, E], MDT, tag="z")
                                nc.vector.tensor_scalar(
                                    zt[:], xall[:, t, :], mv[:, 0:1], rt[:],
                                    OP.subtract, OP.mult,
                                )
                                transpose_block(z1T, zt, t, ps0)
                                # V matmuls for tile t
                                for (c0, cw) in ((0, 512), (512, 256)):
                                    pt2 = ps2.tile([P, 512], F32, tag="mm")
                                    for kc in range(EC):
                                        nc.tensor.matmul(
                                            pt2[:, :cw],
                                            z1T[:, kc, t * P:(t + 1) * P],
                                            wv_sb[:, kc, c0:c0 + cw],
                                            start=(kc == 0), stop=(kc == EC - 1),
                                        )
                                    h0 = c0 // D
                                    nh = cw // D
                                    dst = v_aug[:, t, h0:h0 + nh, 0:D]
                                    if gates["bv"]:
                                        nc.vector.tensor_tensor(
                                            dst,
                                            pt2[:, :cw].rearrange(
                                                "p (h d) -> p h d", d=D),
                                            bv_sb[:, c0:c0 + cw].rearrange(
                                                "p (h d) -> p h d", d=D),
                                            OP.add)
                                    else:
                                        nc.vector.tensor_copy(
                                            out=dst,
                                            in_=pt2[:, :cw].rearrange(
                                                "p (h d) -> p h d", d=D))

                        # ---- ph3: per head pair qk + attention ----
                        # Cycle emission: S^T(all kt) -> PV(q0) -> PV(q1) ->
                        # norms -> qk(c+1).  The qk stream at the cycle end is
                        # 5us of independent PE work during which the next
                        # cycle's psum->SBUF copies and this cycle's norm
                        # chains drain, so the PE re-enters S^T with all
                        # dependencies met.  qk borrows the [P, L] S^T psum
                        # tiles (both lc halves in one tile); PV has its own
                        # 4-bank pool so no intra-cycle WAR aliasing exists.
                        with (
                            tc.tile_pool(name="qkpp", bufs=2) as qkpp,
                            tc.tile_pool(name="wqks", bufs=2) as wqs,
                            tc.tile_pool(name="ptp", bufs=1) as ptp,
                            tc.tile_pool(name="recp", bufs=2) as recp,
                            tc.tile_pool(name="ps3", bufs=4, space="PSUM") as ps3,
                            tc.tile_pool(name="ps3s", bufs=2, space="PSUM") as ps3s,
                        ):
                            # preload the Exp table during qk(0) so the first
                            # real exp doesn't eat a 1.3us table load
                            nc.scalar.activation(tabs[:], tabs[:], AF.Exp)

                            def emit_qk(c):
                                qk_pair = qkpp.tile([P, 2, L], MDT, tag="qkpair",
                                                    name=f"qkp{c}")
                                for i, oc in enumerate((c, EC + c)):
                                    wt = wqs.tile([P, EC, P], MDT, tag="wqk",
                                                  name=f"wqk{c}_{i}")
                                    nc.sync.dma_start(
                                        wt[:], wqkv[:, :, oc * P:(oc + 1) * P])
                                    psums = [ps3.tile([P, 512], F32, tag="mm",
                                                      name=f"qkps{lc}")
                                             for lc in range(QC)]
                                    for kc in range(EC):
                                        for lc in range(QC):
                                            nc.tensor.matmul(
                                                psums[lc][:], wt[:, kc, :],
                                                z1T[:, kc, lc * 512:(lc + 1) * 512],
                                                start=(kc == 0),
                                                stop=(kc == EC - 1),
                                            )
                                    for lc in range(QC):
                                        dst = qk_pair[:, i, lc * 512:(lc + 1) * 512]
                                        if gates["bqk"]:
                                            nc.scalar.activation(
                                                dst, psums[lc][:], AF.Identity,
                                                bias=bqk_sb[:, oc:oc + 1])
                                        else:
                                            # keep these on DVE: a cast on
                                            # the ACT queue sits between exps
                                            # and delays the psum-ring WARs
                                            nc.vector.tensor_copy(
                                                out=dst, in_=psums[lc][:])
                                return qk_pair

                            def emit_st(qk_pair, PTs, kt):
                                # S^T for key tile kt: full row s0:L in one
                                # [P, L] psum tile so same-lhsT matmuls go
                                # back-to-back (pipelined drains), exp split
                                # per query half into PTs[qc][par].
                                s0 = kt * P
                                segs = ([(s0, 512), (512, L)] if s0 < 512
                                        else [(s0, L)])
                                psss = [ps3s.tile([P, L], F32, tag="st",
                                                  name=f"pss{par}")
                                        for par in range(2)]
                                # seg-major emission: the two parities of one
                                # segment are FIFO-adjacent, so their disjoint
                                # PE row groups run concurrently (row packing);
                                # par-major order lets par0's second segment
                                # (same rows as its first) block par1's start.
                                for (a, b) in segs:
                                    for par in range(2):
                                        rows = slice(par * D, par * D + D)
                                        nc.tensor.matmul(
                                            psss[par][:, a:b],
                                            qk_pair[rows, 1, s0:s0 + P],
                                            qk_pair[rows, 0, a:b],
                                            start=True, stop=True)
                                for par in range(2):
                                    pss = psss[par]
                                    if s0 < 512:
                                        nc.scalar.activation(
                                            PTs[0][par][:, kt, s0:512],
                                            pss[:, s0:512], AF.Exp)
                                        nc.scalar.activation(
                                            PTs[1][par][:, kt, 0:512],
                                            pss[:, 512:L], AF.Exp)
                                        nc.vector.tensor_tensor(
                                            PTs[0][par][:, kt, s0:s0 + P],
                                            PTs[0][par][:, kt, s0:s0 + P],
                                            mask_tri, OP.mult)
                                    else:
                                        nc.scalar.activation(
                                            PTs[1][par][:, kt, s0 - 512:512],
                                            pss[:, s0:L], AF.Exp)
                                        nc.vector.tensor_tensor(
                                            PTs[1][par][:, kt,
                                                        s0 - 512:s0 - 512 + P],
                                            PTs[1][par][:, kt,
                                                        s0 - 512:s0 - 512 + P],
                                            mask_tri, OP.mult)

                            def emit_pv_part(c, qc, PTx, psos, kts, first, last):
                                # P@V accumulation sub-group over key tiles
                                # `kts`; the group may be split around other
                                # matmuls (start only on `first`, stop on
                                # `last`).
                                q0 = qc * 512
                                for par in range(2):
                                    h = 2 * c + par
                                    pt_buf = PTx[par]
                                    pso = psos[par]
                                    for idx, j in enumerate(kts):
                                        a = max(j * P, q0)
                                        nc.tensor.matmul(
                                            pso[0:DA, a - q0:512],
                                            v_aug[:, j, h, :],
                                            pt_buf[:, j, a - q0:512],
                                            start=(first and idx == 0),
                                            stop=(last and idx == len(kts) - 1))

                            def emit_pv_norm(c, qc, psos):
                                # psum row 64 carries softmax row-sums (the
                                # ones column of v_aug).  Copy O^T and the
                                # sums row straight out of PSUM so the PV
                                # bank frees after the copies (the bank's
                                # next writer — the coming cycle's qk
                                # matmuls — was measured stalling 1.5us/pair
                                # waiting on the old copy->recip->broadcast->
                                # mult chain); the normalize then runs from
                                # SBUF off the bank-release path.  The
                                # reciprocal input must sit at partition 0
                                # (the custom-DVE seed NaNs on a base-64 row).
                                q0 = qc * 512
                                for par in range(2):
                                    pso = psos[par]
                                    o_rows = slice(par * D, par * D + D)
                                    osb = recp.tile([P, 512], F32, tag="ob")
                                    if qc == 0:
                                        # ACT's queue is exp-free right here
                                        nc.scalar.copy(out=osb[0:D, :],
                                                       in_=pso[0:D, :])
                                    else:
                                        nc.vector.tensor_copy(osb[0:D, :],
                                                              pso[0:D, :])
                                    srow = recp.tile([P, 512], F32, tag="sr")
                                    nc.vector.tensor_copy(srow[0:1, :],
                                                          pso[D:DA, :])
                                    rec = recp.tile([P, 512], F32, tag="rc")
                                    nc.vector.reciprocal_approx_fast(
                                        rec[0:1, :], srow[0:1, :])
                                    recb = recp.tile([P, 512], F32, tag="rb")
                                    nc.gpsimd.partition_broadcast(
                                        recb[0:D, :], rec[0:1, :])
                                    nc.vector.tensor_tensor(
                                        OT[o_rows, c, q0:q0 + 512],
                                        osb[0:D, :], recb[0:D, :], OP.mult,
                                    )

                            # Per-pair cycle, software-pipelined so the PE
                            # never waits on the exp/mask/copy chains:
                            #   S^T(kt0-3) -> qk(c+1) [5us of independent PE
                            #   work while exps+masks drain] -> PV(q0) ->
                            #   PV(q1) over kt0-3 -> S^T(kt4-7) -> PV(q1)
                            #   tail.  The PV(q1) accumulation group stays
                            #   open across the second S^T batch.
                            qkp_cur = emit_qk(0)
                            for c in range(EC):  # head pair (2c, 2c+1)
                                PTs = [[ptp.tile([P, LT, 512], MDT,
                                                 tag=f"pt{qc}{par}",
                                                 name=f"pt{qc}{par}")
                                        for par in range(2)]
                                       for qc in range(QC)]
                                for kt in range(4):
                                    emit_st(qkp_cur, PTs, kt)
                                qkp_next = (emit_qk(c + 1) if c + 1 < EC
                                            else None)
                                psos0 = [ps3.tile([P, 512], F32, tag="mm",
                                                  name=f"pv0{par}")
                                         for par in range(2)]
                                psos1 = [ps3.tile([P, 512], F32, tag="mm",
                                                  name=f"pv1{par}")
                                         for par in range(2)]
                                emit_pv_part(c, 0, PTs[0], psos0,
                                             range(4), True, True)
                                emit_pv_part(c, 1, PTs[1], psos1,
                                             range(4), True, False)
                                for kt in range(4, LT):
                                    emit_st(qkp_cur, PTs, kt)
                                emit_pv_norm(c, 0, psos0)
                                emit_pv_part(c, 1, PTs[1], psos1,
                                             range(4, LT), False, True)
                                emit_pv_norm(c, 1, psos1)
                                qkp_cur = qkp_next
                            # reload the rsqrt table while the PE finishes the
                            # last pair, so ph4's first LN2 chain doesn't wait
                            # the 1.3us load inside the phase-boundary gap
                            nc.scalar.activation(tabs[:], tabs[:],
                                                 AF.Abs_reciprocal_sqrt)

                        # ---- ph4: proj + residual + LN2 + transpose ----
                        z2T = fmp.tile([P, EC, L], MDT, tag="fm", name="z2T")
                        with (
                            tc.tile_pool(name="zp2", bufs=2) as zp2,
                            tc.tile_pool(name="stp2", bufs=2) as stp2,
                            tc.tile_pool(name="ps4", bufs=4, space="PSUM") as ps4,
                            tc.tile_pool(name="ps45", bufs=4, space="PSUM") as ps45,
                        ):
                            # prefetch whole wfc during ph4 (8 chunks so ph5's
                            # first oc doesn't wait on one giant DMA)
                            for j in range(8):
                                nc.sync.dma_start(
                                    wfc_sb[:, :, j * 384:(j + 1) * 384],
                                    wfcv[:, :, j * 384:(j + 1) * 384])
                            prev_z2t = None
                            for t in range(LT):
                                for (c0, cw) in ((0, 512), (512, 256)):
                                    pt = ps4.tile([P, 512], F32, tag="mm")
                                    for kc in range(EC):
                                        nc.tensor.matmul(
                                            pt[:, :cw],
                                            OT[:, kc, t * P:(t + 1) * P],
                                            wproj_sb[:, kc, c0:c0 + cw],
                                            start=(kc == 0), stop=(kc == EC - 1),
                                        )
                                    dst = x1all[:, t, c0:c0 + cw]
                                    if gates["bproj"]:
                                        nc.vector.tensor_tensor(
                                            dst, pt[:, :cw],
                                            bproj_sb[:, c0:c0 + cw], OP.add)
                                        nc.vector.tensor_tensor(
                                            dst, dst, xall[:, t, c0:c0 + cw],
                                            OP.add)
                                    else:
                                        nc.vector.tensor_tensor(
                                            dst, pt[:, :cw],
                                            xall[:, t, c0:c0 + cw], OP.add)
                                mv2, rt2 = ln_tile(x1all[:, t, :], stp2, "2")
                                z2t = zp2.tile([P, E], MDT, tag="z2")
                                nc.vector.tensor_scalar(
                                    z2t[:], x1all[:, t, :], mv2[:, 0:1], rt2[:],
                                    OP.subtract, OP.mult,
                                )
                                # transposes lag one tile so the PE never
                                # waits on the DVE stats/apply chain
                                if prev_z2t is not None:
                                    transpose_block(z2T, prev_z2t, t - 1, ps45)
                                prev_z2t = z2t
                                # fc(lc=0) groups as PE filler: they need only
                                # z2T tiles 0-3 (done after iteration 4) and
                                # absorb the proj ring's DVE-lag stalls.  Raw
                                # psum goes to SBUF via ACT (2D copy); selu
                                # runs in ph5 with the exp table loaded once.
                                if t >= 5:
                                    for g in (2 * (t - 5), 2 * (t - 5) + 1):
                                        pt = ps4.tile([P, 512], F32, tag="mm",
                                                      name=f"fce{g}")
                                        for kc in range(EC):
                                            nc.tensor.matmul(
                                                pt[:],
                                                wfc_sb[:, kc, g * P:(g + 1) * P],
                                                z2T[:, kc, 0:512],
                                                start=(kc == 0),
                                                stop=(kc == EC - 1),
                                            )
                                        nc.scalar.copy(out=raw6[:, g, :],
                                                       in_=pt[:])
                            transpose_block(z2T, prev_z2t, LT - 1, ps45)
                            # preload the exp table for ph5's selu during the
                            # last transposes / first fc matmuls
                            nc.scalar.activation(tabs[:], tabs[:], AF.Exp)

                    # ---- ph5: fc + selu -> hT ----
                    with (
                        tc.tile_pool(name="htp", bufs=1) as htp,
                        tc.tile_pool(name="wop", bufs=1) as wop,
                    ):
                        hT = htp.tile([P, KC2, L], MDT)
                        wo_a = wop.tile([P, KC2, 512], MDT)
                        nc.sync.dma_start(wo_a[:], woutv[:, :, 0:512])
                        wo_b = wop.tile([P, KC2, 256], MDT)
                        nc.sync.dma_start(wo_b[:], woutv[:, :, 512:768])
                        with (
                            tc.tile_pool(name="selu", bufs=2) as slp,
                            # one psum ring spans ph5 AND ph6 so the phase
                            # boundary has no pool-reuse WAR gap
                            tc.tile_pool(name="ps5", bufs=4, space="PSUM") as ps5,
                        ):
                            for oc in range(KC2):
                                for lc in range(QC):
                                    if oc < 6 and lc == 0:
                                        pt = raw6[:, oc, :]
                                    else:
                                        pt = ps5.tile([P, 512], F32, tag="mm")
                                        for kc in range(EC):
                                            nc.tensor.matmul(
                                                pt[:],
                                                wfc_sb[:, kc,
                                                       oc * P:(oc + 1) * P],
                                                z2T[:, kc,
                                                    lc * 512:(lc + 1) * 512],
                                                start=(kc == 0),
                                                stop=(kc == EC - 1),
                                            )
                                    pe_t = slp.tile([P, 512], F32, tag="pe")
                                    bias = (bfce_sb[:, oc:oc + 1] if gates["bfc"]
                                            else lnla_b[:])
                                    nc.scalar.activation(pe_t[:], pt[:], AF.Exp,
                                                         bias=bias,
                                                         scale=1.0 / SELU_LAMBDA)
                                    a_t = slp.tile([P, 512], F32, tag="at")
                                    nc.vector.tensor_scalar(
                                        a_t[:], pe_t[:], SELU_LA, SELU_LA,
                                        OP.min, OP.subtract)
                                    dst = hT[:, oc, lc * 512:(lc + 1) * 512]
                                    if gates["bfc"]:
                                        rl = slp.tile([P, 512], F32, tag="rl")
                                        nc.vector.tensor_scalar(
                                            rl[:], pt[:], bfcl_sb[:, oc:oc + 1],
                                            0.0, OP.add, OP.max)
                                        nc.vector.tensor_tensor(dst, rl[:],
                                                                a_t[:], OP.add)
                                    else:
                                        nc.vector.scalar_tensor_tensor(
                                            dst, pt[:], 0.0, a_t[:],
                                            OP.max, OP.add)

                            # ---- ph6: out = h @ wout + x1 (two passes) ----
                            with tc.tile_pool(name="osp", bufs=3) as osp:
                                ps6 = ps5
                            for t in range(LT):
                                pt = ps6.tile([P, 512], F32, tag="mm")
                                for kc in range(KC2):
                                    nc.tensor.matmul(
                                        pt[:], hT[:, kc, t * P:(t + 1) * P],
                                        wo_a[:, kc, :],
                                        start=(kc == 0), stop=(kc == KC2 - 1),
                                    )
                                ot = osp.tile([P, 512], F32, tag="ot")
                                if gates["bout"]:
                                    nc.vector.tensor_tensor(
                                        ot[:], pt[:], bout_sb[:, 0:512], OP.add)
                                    nc.vector.tensor_tensor(
                                        ot[:], ot[:], x1all[:, t, 0:512], OP.add)
                                else:
                                    nc.vector.tensor_tensor(
                                        ot[:], pt[:], x1all[:, t, 0:512], OP.add)
                                nc.sync.dma_start(outv[:, t, 0:512], ot[:])

                            for t in range(LT):
                                pt = ps6.tile([P, 512], F32, tag="mm")
                                for kc in range(KC2):
                                    nc.tensor.matmul(
                                        pt[:, :256], hT[:, kc, t * P:(t + 1) * P],
                                        wo_b[:, kc, :],
                                        start=(kc == 0), stop=(kc == KC2 - 1),
                                    )
                                ot = osp.tile([P, 512], F32, tag="ot")
                                if gates["bout"]:
                                    nc.vector.tensor_tensor(
                                        ot[:, :256], pt[:, :256],
                                        bout_sb[:, 512:768], OP.add)
                                    nc.vector.tensor_tensor(
                                        ot[:, :256], ot[:, :256],
                                        x1all[:, t, 512:768], OP.add)
                                else:
                                    nc.vector.tensor_tensor(
                                        ot[:, :256], pt[:, :256],
                                        x1all[:, t, 512:768], OP.add)
                                nc.sync.dma_start(outv[:, t, 512:768],
                                                  ot[:, :256])

    nc.finalize()
    return nc


def kernel(**inputs):
    global _last_results

    mm_dt_name = os.environ.get("KERNEL_MM_DT", "bf16")

    def arr(name):
        return np.ascontiguousarray(np.asarray(inputs[name], dtype=np.float32))

    x = arr("x")                       # [8, 1024, 768]
    g1 = arr("ln1_scale")
    b1 = arr("ln1_bias")
    w_qkv = arr("w_qkv")               # [768, 2304]
    b_qkv = arr("b_qkv")
    w_proj = arr("w_proj")
    b_proj = arr("b_proj")
    g2 = arr("ln2_scale")
    b2 = arr("ln2_bias")
    w_fc = arr("w_fc")
    b_fc = arr("b_fc")
    w_out = arr("w_out")
    b_out = arr("b_out")

    qscale = np.float32(1.0 / np.sqrt(D))

    w3 = w_qkv.reshape(E, H, 3, D)
    qw = (w3[:, :, 0, :].reshape(E, E) * qscale)
    kw = w3[:, :, 1, :].reshape(E, E)
    vw = w3[:, :, 2, :].reshape(E, E)
    wqk = np.ascontiguousarray(
        np.concatenate([qw, kw], axis=1) * g1[:, None]).astype(np.float32)
    wv = np.ascontiguousarray(vw * g1[:, None]).astype(np.float32)

    bq3 = (b1 @ w_qkv + b_qkv).reshape(H, 3, D)
    bqk = np.concatenate(
        [bq3[:, 0, :].reshape(E) * qscale, bq3[:, 1, :].reshape(E)]).astype(np.float32)
    bv = np.ascontiguousarray(bq3[:, 2, :].reshape(E)).astype(np.float32)

    wfc_p = np.ascontiguousarray(
        w_fc * g2[:, None] * np.float32(SELU_LAMBDA)).astype(np.float32)
    bfc_eff = (b2 @ w_fc + b_fc).astype(np.float32)
    bfce = (bfc_eff + np.float32(np.log(SELU_LA))).astype(np.float32)
    bfcl = (bfc_eff * np.float32(SELU_LAMBDA)).astype(np.float32)

    gates = {
        "bqk": bool(np.any(bqk != 0)),
        "bv": bool(np.any(bv != 0)),
        "bproj": bool(np.any(b_proj != 0)),
        "bfc": bool(np.any(bfc_eff != 0)),
        "bout": bool(np.any(b_out != 0)),
    }

    key = (tuple(sorted(gates.items())), mm_dt_name)
    if key not in _build_cache:
        _build_cache[key] = _build(gates, mm_dt_name)
    nc = _build_cache[key]

    wdt = np.float32 if mm_dt_name == "f32r" else ml_dtypes.bfloat16

    def wcast(a):
        return np.ascontiguousarray(a.astype(wdt))

    base = {
        "wqk": wcast(wqk), "wv": wcast(wv),
        "wproj": wcast(w_proj),
        "wfc": wcast(wfc_p),
        "wout": wcast(w_out),
    }
    if gates["bqk"]:
        base["bqk"] = bqk
    if gates["bv"]:
        base["bv"] = bv
    if gates["bproj"]:
        base["bproj"] = np.ascontiguousarray(b_proj)
    if gates["bfc"]:
        base["bfce"] = bfce
        base["bfcl"] = bfcl
    if gates["bout"]:
        base["bout"] = np.ascontiguousarray(b_out)

    in_maps = [dict(base, x=np.ascontiguousarray(x[c])) for c in range(NCORES)]
    res = bass_utils.run_bass_kernel_spmd(nc, in_maps, core_ids=list(range(NCORES)))
    _last_results = res
    out = np.stack([res.results[c]["out"] for c in range(NCORES)], axis=0)
    return out.astype(np.float32)


# revision 57
# speedup vs baseline: 1.1674x; 1.1674x over previous
"""Trainium2 Bass kernel for nn_AttentionBlock_68624987455817.

Pre-LN causal self-attention block + MLP (B=8, L=1024, E=768, H=12, D=64).

Sharding: data-parallel over batch B=8 across the 8 NeuronCores (one batch
element per core, weights replicated, no collectives). Each core runs the
full block on its [1024, 768] slice.

Optimized from the first working kernel (391us) to ~335us. The changes that
mattered, in order of impact:
  - all transposes on the PE (identity matmul) — the DMA-transpose path left
    the PE idle ~100us across the LN1 prologue and the LN2 mid-kernel valley.
  - per-tile LN stats (bn_stats/bn_aggr, stats emitted 3 tiles ahead) so each
    token tile's stats -> apply -> transpose -> matmul chain pipelines
    instead of barriering on all 8 tiles; LN1 fused with the V matmuls.
  - x1 (attention residual) stays in SBUF; the DRAM store/reload roundtrip
    and the separate LN2 pass are gone. proj -> LN2 -> transpose is
    software-pipelined by one tile so the PE never waits on the DVE chain.
  - ph3 software pipeline (see below): the exp/mask/copy chains of pair c
    complete under pair c+1's qk matmul stream; PV(q1) accumulation is split
    around the second S^T batch so it never waits on fresh exps.  Without
    this the recurring sub-us PE stalls also re-trigger the HAM clock gate
    (PE drops to 1.2 GHz) — the stall cost roughly doubles itself.
  - matmul operand dtype defaults to bf16 (1 cyc/col + FWL weight loads);
    rel err ~3.2e-3 vs the 2e-2 gate.  Counterintuitively bf16 alone was
    NOT faster than f32r (f32r also streams ~1 col/cycle warm); the wins
    came from the scheduling changes above.
  - whole wfc prefetched during ph4, wout passes A+B during ph5, wv/wproj
    during ph0; x tiles 0-1 DMA'd before any setup; ACT function tables
    (rsqrt/exp) preloaded off the critical path (a table switch is 1.3us).
  - engine placement tuned: psum->SBUF copies split across Vector/Scalar,
    masks+normalize on Vector, partition-broadcast on GpSimd.  GpSimd is
    ONLY safe for ops whose consumers are far away (its strict FIFO +
    per-op library reload otherwise serialize the whole pipeline — moving
    masks or LN applies there cost 2x overall).

Per-core dataflow (activations feature-major through the matmuls):
  ph0+2 per token tile t: LN1 stats (bn_stats) -> z tile -> PE transpose
        into z1T -> V matmuls (ones column per head makes P@V emit softmax
        row-sums at psum row 64).
  ph3   per head pair (2c, 2c+1), cycle: S^T(kt0-3) -> qk(c+1) -> PV(q0) ->
        PV(q1) over kt0-3 -> S^T(kt4-7) -> PV(q1) tail -> normalize.
        S^T = k_h^T q_h in [P, L] psums (seg-major emission so the two
        parities' disjoint PE row groups pack); exp -> P^T (masked);
        [O^T; sums] = Vaug^T P^T; normalize via fast reciprocal + gpsimd
        partition broadcast.
  ph4   per tile: x1 = O @ wproj + x (SBUF-resident); LN2 stats -> z2 ->
        PE transpose into z2T (pipelined one tile behind the proj matmuls).
  ph5   hT = selu(wfc^T @ z2T)  (wfc pre-scaled by selu lambda)
  ph6   out = h @ wout + x1     (two column passes; wout prefetched in ph5)

LN scales fold into the following weight matrices host-side; LN biases and
all linear biases fold into per-feature biases only materialized on-chip
when nonzero (all zero for this problem's inputs).
"""
import os
import sys

sys.path.insert(0, "/opt/trn_rl_repo")

import numpy as np
import ml_dtypes

import concourse.bass as bass
from concourse import bacc
import concourse.mybir as mybir
from concourse.tile import TileContext
from concourse import bass_utils
from concourse.masks import make_identity

F32 = mybir.dt.float32
F32R = mybir.dt.float32r
BF16 = mybir.dt.bfloat16
AF = mybir.ActivationFunctionType
OP = mybir.AluOpType
AX = mybir.AxisListType

P = 128
L = 1024
E = 768
H = 12
D = 64
DA = D + 1           # V columns + ones column (row-sum trick)
EC = E // P          # 6 feature chunks
LT = L // P          # 8 token tiles
QC = L // 512        # 2 query chunks
KC2 = 4 * E // P     # 24 chunks of the MLP hidden dim
NCORES = 8

SELU_LAMBDA = 1.0507009873554805
SELU_ALPHA = 1.6732632423543772
SELU_LA = SELU_LAMBDA * SELU_ALPHA
LN_EPS = 1e-6

_last_results = None
_build_cache = {}


def _build(gates, mm_dt_name):
    MDT = {"f32r": F32R, "bf16": BF16}[mm_dt_name]
    PSDT = MDT  # transpose output dtype must match its input dtype

    nc = bacc.Bacc("TRN2", target_bir_lowering=False)

    x_d = nc.dram_tensor("x", [L, E], F32, kind="ExternalInput")
    wqk_d = nc.dram_tensor("wqk", [E, 2 * E], MDT, kind="ExternalInput")
    wv_d = nc.dram_tensor("wv", [E, E], MDT, kind="ExternalInput")
    wproj_d = nc.dram_tensor("wproj", [E, E], MDT, kind="ExternalInput")
    wfc_d = nc.dram_tensor("wfc", [E, 4 * E], MDT, kind="ExternalInput")
    wout_d = nc.dram_tensor("wout", [4 * E, E], MDT, kind="ExternalInput")
    out_d = nc.dram_tensor("out", [L, E], F32, kind="ExternalOutput")

    bqk_d = bv_d = bproj_d = bfce_d = bfcl_d = bout_d = None
    if gates["bqk"]:
        bqk_d = nc.dram_tensor("bqk", [2 * E], F32, kind="ExternalInput")
    if gates["bv"]:
        bv_d = nc.dram_tensor("bv", [E], F32, kind="ExternalInput")
    if gates["bproj"]:
        bproj_d = nc.dram_tensor("bproj", [E], F32, kind="ExternalInput")
    if gates["bfc"]:
        bfce_d = nc.dram_tensor("bfce", [4 * E], F32, kind="ExternalInput")
        bfcl_d = nc.dram_tensor("bfcl", [4 * E], F32, kind="ExternalInput")
    if gates["bout"]:
        bout_d = nc.dram_tensor("bout", [E], F32, kind="ExternalInput")

    xv = x_d.rearrange("(t p) e -> p t e", p=P)            # [128, 8, 768]
    wqkv = wqk_d.rearrange("(c p) m -> p c m", p=P)        # [128, 6, 1536]
    wvv = wv_d.rearrange("(c p) m -> p c m", p=P)          # [128, 6, 768]
    wprojv = wproj_d.rearrange("(c p) m -> p c m", p=P)    # [128, 6, 768]
    wfcv = wfc_d.rearrange("(c p) m -> p c m", p=P)        # [128, 6, 3072]
    woutv = wout_d.rearrange("(c p) m -> p c m", p=P)      # [128, 24, 768]
    outv = out_d.rearrange("(t p) e -> p t e", p=P)

    with TileContext(nc) as tc:
        with tc.tile_pool(name="pers", bufs=1) as pers:
            xall = pers.tile([P, LT, E], F32)    # x tiles, live ph0 -> ph4
            x1all = pers.tile([P, LT, E], F32)   # x1 tiles, live ph4 -> ph6
            # x tiles 0-1 gate the whole LN1 chain: issue their DMAs before
            # any setup so the transfer overlaps the constant initialization.
            for t in range(2):
                nc.sync.dma_start(xall[:, t, :], xv[:, t, :])
            # preload the rsqrt ACT table while the DMAs run so the first LN
            # chain doesn't eat the 1.3us table load; the exp table is
            # preloaded at ph3 entry (ACT holds one table at a time).
            tabs = pers.tile([P, 1], F32)
            nc.vector.memset(tabs[:], 0.5)
            nc.scalar.activation(tabs[:], tabs[:], AF.Abs_reciprocal_sqrt)

            # mask_tri[p, f] = 1.0 if f >= p else 0.0 (keep where k <= q).
            # Built in f32 (f32r memset/affine_select fail walrus codegen).
            mask_f32 = pers.tile([P, P], F32)
            nc.gpsimd.memset(mask_f32[:], 0.0)
            nc.gpsimd.affine_select(
                out=mask_f32[:], in_=mask_f32[:],
                compare_op=OP.is_ge, fill=1.0, base=-1,
                pattern=[[-1, P]], channel_multiplier=1,
            )
            if MDT == F32R:
                mask_tri = mask_f32[:].bitcast(F32R)
            else:
                mask_b = pers.tile([P, P], BF16)
                nc.vector.tensor_copy(mask_b[:], mask_f32[:])
                mask_tri = mask_b[:]
            ones_f32 = pers.tile([P, LT * H], F32)
            nc.vector.memset(ones_f32[:], 1.0)
            eps_b = pers.tile([P, 1], F32)
            nc.vector.memset(eps_b[:], LN_EPS)
            lnla_b = pers.tile([P, 1], F32)
            nc.vector.memset(lnla_b[:], float(np.log(SELU_LA)))

            ident = pers.tile([P, P], F32)
            make_identity(nc, ident)
            ident_m = pers.tile([P, P], MDT)
            nc.vector.tensor_copy(ident_m[:], ident[:])

            bqk_sb = bv_sb = bproj_sb = bfce_sb = bfcl_sb = bout_sb = None
            if gates["bqk"]:
                bqk_sb = pers.tile([P, 2 * EC], F32)
                nc.sync.dma_start(bqk_sb[:], bqk_d.rearrange("(c p) -> p c", p=P))
            if gates["bv"]:
                bv_sb = pers.tile([P, E], F32)
                nc.sync.dma_start(bv_sb[:], bv_d.to_broadcast((P, E)))
            if gates["bproj"]:
                bproj_sb = pers.tile([P, E], F32)
                nc.sync.dma_start(bproj_sb[:], bproj_d.to_broadcast((P, E)))
            if gates["bfc"]:
                bfce_sb = pers.tile([P, KC2], F32)
                nc.sync.dma_start(bfce_sb[:], bfce_d.rearrange("(c p) -> p c", p=P))
                bfcl_sb = pers.tile([P, KC2], F32)
                nc.sync.dma_start(bfcl_sb[:], bfcl_d.rearrange("(c p) -> p c", p=P))
            if gates["bout"]:
                bout_sb = pers.tile([P, E], F32)
                nc.sync.dma_start(bout_sb[:], bout_d.to_broadcast((P, E)))

            def transpose_block(dstT, src_tile, t, pspool):
                """dstT[:, c, t*P:(t+1)*P] = src_tile[:, c*P:(c+1)*P].T"""
                for c in range(EC):
                    pt = pspool.tile([P, P], PSDT, tag="tr")
                    nc.tensor.transpose(pt[:], src_tile[:, c * P:(c + 1) * P],
                                        ident_m[:])
                    nc.scalar.copy(out=dstT[:, c, t * P:(t + 1) * P],
                                   in_=pt[:])

            def ln_tile(src, stpool, tag):
                """bn_stats mean/var over the free axis + rsqrt(var+eps)."""
                bnst = stpool.tile([P, 2, 6], F32, tag=f"bn{tag}", name="bnst")
                xg = src.rearrange("p (n f) -> p n f", f=384)
                for g in range(2):
                    nc.vector.bn_stats(bnst[:, g, :], xg[:, g, :])
                mv = stpool.tile([P, 2], F32, tag=f"mv{tag}", name="mv")
                nc.vector.bn_aggr(mv[:], bnst[:])
                rt = stpool.tile([P, 1], F32, tag=f"rt{tag}", name="rt")
                nc.scalar.activation(rt[:], mv[:, 1:2], AF.Abs_reciprocal_sqrt,
                                     bias=eps_b[:])
                return mv, rt

            with tc.tile_pool(name="fm", bufs=1) as fmp:
                z1T = fmp.tile([P, EC, L], MDT, tag="fm", name="z1T")
                with tc.tile_pool(name="wfcp", bufs=1) as wfcp:
                    wfc_sb = wfcp.tile([P, EC, 4 * E], MDT)
                    # raw pre-selu outputs of the fc groups computed early
                    # inside ph4 (selu applied in ph5 to avoid rsqrt/exp
                    # ACT-table thrash)
                    raw6 = wfcp.tile([P, 6, 512], BF16)
                    with (
                        tc.tile_pool(name="otp", bufs=1) as otp,
                        tc.tile_pool(name="vp", bufs=1) as vpool,
                        tc.tile_pool(name="wpp", bufs=1) as wpp,
                    ):
                        OT = otp.tile([P, EC, L], MDT)
                        v_aug = vpool.tile([P, LT, H, DA], MDT)
                        wproj_sb = wpp.tile([P, EC, E], MDT)

                        # ---- ph0+ph2 fused: per-tile LN1 -> z1T -> V ----
                        with (
                            tc.tile_pool(name="wvp", bufs=1) as wvp,
                            tc.tile_pool(name="zp", bufs=2) as zp,
                            tc.tile_pool(name="stp", bufs=4) as stp,
                            tc.tile_pool(name="ps0", bufs=4, space="PSUM") as ps0,
                            tc.tile_pool(name="ps2", bufs=4, space="PSUM") as ps2,
                        ):
                            wv_sb = wvp.tile([P, EC, E], MDT)
                            nc.vector.tensor_copy(
                                v_aug[:, :, :, D:DA],
                                ones_f32[:].rearrange("p (t h o) -> p t h o",
                                                      h=H, o=1))
                            # x tiles 0-1 already in flight (issued at pers
                            # setup); wv next (needed by V at ~7us), then the
                            # rest of x, then wproj.
                            nc.sync.dma_start(wv_sb[:], wvv[:])
                            for t in range(2, LT):
                                nc.sync.dma_start(xall[:, t, :], xv[:, t, :])
                            nc.sync.dma_start(wproj_sb[:], wprojv[:])
                            # LN1 stats run 3 tiles ahead so the ACT rsqrt is
                            # never queued behind transpose copies.
                            stats1 = {}
                            for t in range(3):
                                stats1[t] = ln_tile(xall[:, t, :], stp, "1")
                            for t in range(LT):
                                if t + 3 < LT:
                                    stats1[t + 3] = ln_tile(
                                        xall[:, t + 3, :], stp, "1")
                                mv, rt = stats1.pop(t)
                                zt = zp.tile([P, E], MDT, tag="z")
                                nc.vector.tensor_scalar(
                                    zt[:], xall[:, t, :], mv[:, 0:1], rt[:],
                                    OP.subtract, OP.mult,
                                )
                                transpose_block(z1T, zt, t, ps0)
                                # V matmuls for tile t
                                for (c0, cw) in ((0, 512), (512, 256)):
                                    pt2 = ps2.tile([P, 512], F32, tag="mm")
                                    for kc in range(EC):
                                        nc.tensor.matmul(
                                            pt2[:, :cw],
                                            z1T[:, kc, t * P:(t + 1) * P],
                                            wv_sb[:, kc, c0:c0 + cw],
                                            start=(kc == 0), stop=(kc == EC - 1),
                                        )
                                    h0 = c0 // D
                                    nh = cw // D
                                    dst = v_aug[:, t, h0:h0 + nh, 0:D]
                                    if gates["bv"]:
                                        nc.vector.tensor_tensor(
                                            dst,
                                            pt2[:, :cw].rearrange(
                                                "p (h d) -> p h d", d=D),
                                            bv_sb[:, c0:c0 + cw].rearrange(
                                                "p (h d) -> p h d", d=D),
                                            OP.add)
                                    else:
                                        nc.vector.tensor_copy(
                                            out=dst,
                                            in_=pt2[:, :cw].rearrange(
                                                "p (h d) -> p h d", d=D))

                        # ---- ph3: per head pair qk + attention ----
                        # Cycle emission: S^T(all kt) -> PV(q0) -> PV(q1) ->
                        # norms -> qk(c+1).  The qk stream at the cycle end is
                        # 5us of independent PE work during which the next
                        # cycle's psum->SBUF copies and this cycle's norm
                        # chains drain, so the PE re-enters S^T with all
                        # dependencies met.  qk borrows the [P, L] S^T psum
                        # tiles (both lc halves in one tile); PV has its own
                        # 4-bank pool so no intra-cycle WAR aliasing exists.
                        with (
                            tc.tile_pool(name="qkpp", bufs=2) as qkpp,
                            tc.tile_pool(name="wqks", bufs=2) as wqs,
                            tc.tile_pool(name="ptp", bufs=1) as ptp,
                            tc.tile_pool(name="recp", bufs=2) as recp,
                            tc.tile_pool(name="ps3", bufs=4, space="PSUM") as ps3,
                            tc.tile_pool(name="ps3s", bufs=2, space="PSUM") as ps3s,
                        ):
                            # preload the Exp table during qk(0) so the first
                            # real exp doesn't eat a 1.3us table load
                            nc.scalar.activation(tabs[:], tabs[:], AF.Exp)

                            def emit_qk(c):
                                qk_pair = qkpp.tile([P, 2, L], MDT, tag="qkpair",
                                                    name=f"qkp{c}")
                                for i, oc in enumerate((c, EC + c)):
                                    wt = wqs.tile([P, EC, P], MDT, tag="wqk",
                                                  name=f"wqk{c}_{i}")
                                    nc.sync.dma_start(
                                        wt[:], wqkv[:, :, oc * P:(oc + 1) * P])
                                    psums = [ps3.tile([P, 512], F32, tag="mm",
                                                      name=f"qkps{lc}")
                                             for lc in range(QC)]
                                    for kc in range(EC):
                                        for lc in range(QC):
                                            nc.tensor.matmul(
                                                psums[lc][:], wt[:, kc, :],
                                                z1T[:, kc, lc * 512:(lc + 1) * 512],
                                                start=(kc == 0),
                                                stop=(kc == EC - 1),
                                            )
                                    for lc in range(QC):
                                        dst = qk_pair[:, i, lc * 512:(lc + 1) * 512]
                                        if gates["bqk"]:
                                            nc.scalar.activation(
                                                dst, psums[lc][:], AF.Identity,
                                                bias=bqk_sb[:, oc:oc + 1])
                                        else:
                                            # keep these on DVE: a cast on
                                            # the ACT queue sits between exps
                                            # and delays the psum-ring WARs
                                            nc.vector.tensor_copy(
                                                out=dst, in_=psums[lc][:])
                                return qk_pair

                            def emit_st(qk_pair, PTs, kt):
                                # S^T for key tile kt: full row s0:L in one
                                # [P, L] psum tile so same-lhsT matmuls go
                                # back-to-back (pipelined drains), exp split
                                # per query half into PTs[qc][par].
                                s0 = kt * P
                                segs = ([(s0, 512), (512, L)] if s0 < 512
                                        else [(s0, L)])
                                psss = [ps3s.tile([P, L], F32, tag="st",
                                                  name=f"pss{par}")
                                        for par in range(2)]
                                # seg-major emission: the two parities of one
                                # segment are FIFO-adjacent, so their disjoint
                                # PE row groups run concurrently (row packing);
                                # par-major order lets par0's second segment
                                # (same rows as its first) block par1's start.
                                for (a, b) in segs:
                                    for par in range(2):
                                        rows = slice(par * D, par * D + D)
                                        nc.tensor.matmul(
                                            psss[par][:, a:b],
                                            qk_pair[rows, 1, s0:s0 + P],
                                            qk_pair[rows, 0, a:b],
                                            start=True, stop=True)
                                for par in range(2):
                                    pss = psss[par]
                                    if s0 < 512:
                                        nc.scalar.activation(
                                            PTs[0][par][:, kt, s0:512],
                                            pss[:, s0:512], AF.Exp)
                                        nc.scalar.activation(
                                            PTs[1][par][:, kt, 0:512],
                                            pss[:, 512:L], AF.Exp)
                                        nc.vector.tensor_tensor(
                                            PTs[0][par][:, kt, s0:s0 + P],
                                            PTs[0][par][:, kt, s0:s0 + P],
                                            mask_tri, OP.mult)
                                    else:
                                        nc.scalar.activation(
                                            PTs[1][par][:, kt, s0 - 512:512],
                                            pss[:, s0:L], AF.Exp)
                                        nc.vector.tensor_tensor(
                                            PTs[1][par][:, kt,
                                                        s0 - 512:s0 - 512 + P],
                                            PTs[1][par][:, kt,
                                                        s0 - 512:s0 - 512 + P],
                                            mask_tri, OP.mult)

                            def emit_pv_part(c, qc, PTx, psos, kts, first, last):
                                # P@V accumulation sub-group over key tiles
                                # `kts`; the group may be split around other
                                # matmuls (start only on `first`, stop on
                                # `last`).
                                q0 = qc * 512
                                for par in range(2):
                                    h = 2 * c + par
                                    pt_buf = PTx[par]
                                    pso = psos[par]
                                    for idx, j in enumerate(kts):
                                        a = max(j * P, q0)
                                        nc.tensor.matmul(
                                            pso[0:DA, a - q0:512],
                                            v_aug[:, j, h, :],
                                            pt_buf[:, j, a - q0:512],
                                            start=(first and idx == 0),
                                            stop=(last and idx == len(kts) - 1))

                            def emit_pv_norm(c, qc, psos):
                                # psum row 64 carries softmax row-sums (the
                                # ones column of v_aug).  Copy O^T and the
                                # sums row straight out of PSUM so the PV
                                # bank frees after the copies (the bank's
                                # next writer — the coming cycle's qk
                                # matmuls — was measured stalling 1.5us/pair
                                # waiting on the old copy->recip->broadcast->
                                # mult chain); the normalize then runs from
                                # SBUF off the bank-release path.  The
                                # reciprocal input must sit at partition 0
                                # (the custom-DVE seed NaNs on a base-64 row).
                                q0 = qc * 512
                                for par in range(2):
                                    pso = psos[par]
                                    o_rows = slice(par * D, par * D + D)
                                    osb = recp.tile([P, 512], F32, tag="ob")
                                    if qc == 0:
                                        # ACT's queue is exp-free right here
                                        nc.scalar.copy(out=osb[0:D, :],
                                                       in_=pso[0:D, :])
                                    else:
                                        nc.vector.tensor_copy(osb[0:D, :],
                                                              pso[0:D, :])
                                    srow = recp.tile([P, 512], F32, tag="sr")
                                    nc.vector.tensor_copy(srow[0:1, :],
                                                          pso[D:DA, :])
                                    rec = recp.tile([P, 512], F32, tag="rc")
                                    nc.vector.reciprocal_approx_fast(
                                        rec[0:1, :], srow[0:1, :])
                                    recb = recp.tile([P, 512], F32, tag="rb")
                                    nc.gpsimd.partition_broadcast(
                                        recb[0:D, :], rec[0:1, :])
                                    nc.vector.tensor_tensor(
                                        OT[o_rows, c, q0:q0 + 512],
                                        osb[0:D, :], recb[0:D, :], OP.mult,
                                    )

                            # Per-pair cycle, software-pipelined so the PE
                            # never waits on the exp/mask/copy chains:
                            #   S^T(kt0-3) -> qk(c+1) [5us of independent PE
                            #   work while exps+masks drain] -> PV(q0) ->
                            #   PV(q1) over kt0-3 -> S^T(kt4-7) -> PV(q1)
                            #   tail.  The PV(q1) accumulation group stays
                            #   open across the second S^T batch.
                            qkp_cur = emit_qk(0)
                            # prefetch whole wfc NOW: ph3 has ~130us of idle
                            # DMA bandwidth, and ph4's interleaved fc groups
                            # need chunks 0-1 long before ph5 (a late wfc
                            # DMA head-of-line blocks the in-order PE FIFO)
                            for j in range(8):
                                nc.sync.dma_start(
                                    wfc_sb[:, :, j * 384:(j + 1) * 384],
                                    wfcv[:, :, j * 384:(j + 1) * 384])
                            for c in range(EC):  # head pair (2c, 2c+1)
                                PTs = [[ptp.tile([P, LT, 512], MDT,
                                                 tag=f"pt{qc}{par}",
                                                 name=f"pt{qc}{par}")
                                        for par in range(2)]
                                       for qc in range(QC)]
                                for kt in range(4):
                                    emit_st(qkp_cur, PTs, kt)
                                qkp_next = (emit_qk(c + 1) if c + 1 < EC
                                            else None)
                                psos0 = [ps3.tile([P, 512], F32, tag="mm",
                                                  name=f"pv0{par}")
                                         for par in range(2)]
                                psos1 = [ps3.tile([P, 512], F32, tag="mm",
                                                  name=f"pv1{par}")
                                         for par in range(2)]
                                emit_pv_part(c, 0, PTs[0], psos0,
                                             range(4), True, True)
                                emit_pv_part(c, 1, PTs[1], psos1,
                                             range(4), True, False)
                                for kt in range(4, LT):
                                    emit_st(qkp_cur, PTs, kt)
                                emit_pv_norm(c, 0, psos0)
                                emit_pv_part(c, 1, PTs[1], psos1,
                                             range(4, LT), False, True)
                                emit_pv_norm(c, 1, psos1)
                                qkp_cur = qkp_next
                            # reload the rsqrt table while the PE finishes the
                            # last pair, so ph4's first LN2 chain doesn't wait
                            # the 1.3us load inside the phase-boundary gap
                            nc.scalar.activation(tabs[:], tabs[:],
                                                 AF.Abs_reciprocal_sqrt)

                        # ---- ph4: proj + residual + LN2 + transpose ----
                        z2T = fmp.tile([P, EC, L], MDT, tag="fm", name="z2T")
                        with (
                            tc.tile_pool(name="zp2", bufs=2) as zp2,
                            tc.tile_pool(name="stp2", bufs=2) as stp2,
                            tc.tile_pool(name="ps4", bufs=4, space="PSUM") as ps4,
                            tc.tile_pool(name="ps45", bufs=4, space="PSUM") as ps45,
                        ):
                            prev_z2t = None
                            for t in range(LT):
                                for (c0, cw) in ((0, 512), (512, 256)):
                                    pt = ps4.tile([P, 512], F32, tag="mm")
                                    for kc in range(EC):
                                        nc.tensor.matmul(
                                            pt[:, :cw],
                                            OT[:, kc, t * P:(t + 1) * P],
                                            wproj_sb[:, kc, c0:c0 + cw],
                                            start=(kc == 0), stop=(kc == EC - 1),
                                        )
                                    dst = x1all[:, t, c0:c0 + cw]
                                    if gates["bproj"]:
                                        nc.vector.tensor_tensor(
                                            dst, pt[:, :cw],
                                            bproj_sb[:, c0:c0 + cw], OP.add)
                                        nc.vector.tensor_tensor(
                                            dst, dst, xall[:, t, c0:c0 + cw],
                                            OP.add)
                                    else:
                                        nc.vector.tensor_tensor(
                                            dst, pt[:, :cw],
                                            xall[:, t, c0:c0 + cw], OP.add)
                                mv2, rt2 = ln_tile(x1all[:, t, :], stp2, "2")
                                z2t = zp2.tile([P, E], MDT, tag="z2")
                                nc.vector.tensor_scalar(
                                    z2t[:], x1all[:, t, :], mv2[:, 0:1], rt2[:],
                                    OP.subtract, OP.mult,
                                )
                                # transposes lag one tile so the PE never
                                # waits on the DVE stats/apply chain
                                if prev_z2t is not None:
                                    transpose_block(z2T, prev_z2t, t - 1, ps45)
                                prev_z2t = z2t
                                # fc(lc=0) groups as PE filler: they need only
                                # z2T tiles 0-3 (done after iteration 4) and
                                # the ph3-prefetched wfc; they absorb the proj
                                # ring's DVE-lag stalls.  Raw psum -> SBUF via
                                # ACT 2D copy; selu runs in ph5 with the exp
                                # table loaded once.
                                if t >= 5:
                                    for g in (2 * (t - 5), 2 * (t - 5) + 1):
                                        pt = ps4.tile([P, 512], F32, tag="mm",
                                                      name=f"fce{g}")
                                        for kc in range(EC):
                                            nc.tensor.matmul(
                                                pt[:],
                                                wfc_sb[:, kc, g * P:(g + 1) * P],
                                                z2T[:, kc, 0:512],
                                                start=(kc == 0),
                                                stop=(kc == EC - 1),
                                            )
                                        nc.scalar.copy(out=raw6[:, g, :],
                                                       in_=pt[:])
                            transpose_block(z2T, prev_z2t, LT - 1, ps45)
                            # preload the exp table for ph5's selu during the
                            # last transposes / first fc matmuls
                            nc.scalar.activation(tabs[:], tabs[:], AF.Exp)

                    # ---- ph5: fc + selu -> hT ----
                    with (
                        tc.tile_pool(name="htp", bufs=1) as htp,
                        tc.tile_pool(name="wop", bufs=1) as wop,
                    ):
                        hT = htp.tile([P, KC2, L], MDT)
                        wo_a = wop.tile([P, KC2, 512], MDT)
                        nc.sync.dma_start(wo_a[:], woutv[:, :, 0:512])
                        wo_b = wop.tile([P, KC2, 256], MDT)
                        nc.sync.dma_start(wo_b[:], woutv[:, :, 512:768])
                        with (
                            tc.tile_pool(name="selu", bufs=2) as slp,
                            # one psum ring spans ph5 AND ph6 so the phase
                            # boundary has no pool-reuse WAR gap
                            tc.tile_pool(name="ps5", bufs=4, space="PSUM") as ps5,
                        ):
                            for oc in range(KC2):
                                for lc in range(QC):
                                    if oc < 6 and lc == 0:
                                        pt = raw6[:, oc, :]
                                    else:
                                        pt = ps5.tile([P, 512], F32, tag="mm")
                                        for kc in range(EC):
                                            nc.tensor.matmul(
                                                pt[:],
                                                wfc_sb[:, kc,
                                                       oc * P:(oc + 1) * P],
                                                z2T[:, kc,
                                                    lc * 512:(lc + 1) * 512],
                                                start=(kc == 0),
                                                stop=(kc == EC - 1),
                                            )
                                    pe_t = slp.tile([P, 512], F32, tag="pe")
                                    bias = (bfce_sb[:, oc:oc + 1] if gates["bfc"]
                                            else lnla_b[:])
                                    nc.scalar.activation(pe_t[:], pt[:], AF.Exp,
                                                         bias=bias,
                                                         scale=1.0 / SELU_LAMBDA)
                                    a_t = slp.tile([P, 512], F32, tag="at")
                                    nc.vector.tensor_scalar(
                                        a_t[:], pe_t[:], SELU_LA, SELU_LA,
                                        OP.min, OP.subtract)
                                    dst = hT[:, oc, lc * 512:(lc + 1) * 512]
                                    if gates["bfc"]:
                                        rl = slp.tile([P, 512], F32, tag="rl")
                                        nc.vector.tensor_scalar(
                                            rl[:], pt[:], bfcl_sb[:, oc:oc + 1],
                                            0.0, OP.add, OP.max)
                                        nc.vector.tensor_tensor(dst, rl[:],
                                                                a_t[:], OP.add)
                                    else:
                                        nc.vector.scalar_tensor_tensor(
                                            dst, pt[:], 0.0, a_t[:],
                                            OP.max, OP.add)

                            # ---- ph6: out = h @ wout + x1 (two passes) ----
                            with tc.tile_pool(name="osp", bufs=3) as osp:
                                ps6 = ps5
                            for t in range(LT):
                                pt = ps6.tile([P, 512], F32, tag="mm")
                                for kc in range(KC2):
                                    nc.tensor.matmul(
                                        pt[:], hT[:, kc, t * P:(t + 1) * P],
                                        wo_a[:, kc, :],
                                        start=(kc == 0), stop=(kc == KC2 - 1),
                                    )
                                ot = osp.tile([P, 512], F32, tag="ot")
                                if gates["bout"]:
                                    nc.vector.tensor_tensor(
                                        ot[:], pt[:], bout_sb[:, 0:512], OP.add)
                                    nc.vector.tensor_tensor(
                                        ot[:], ot[:], x1all[:, t, 0:512], OP.add)
                                else:
                                    nc.vector.tensor_tensor(
                                        ot[:], pt[:], x1all[:, t, 0:512], OP.add)
                                nc.sync.dma_start(outv[:, t, 0:512], ot[:])

                            for t in range(LT):
                                pt = ps6.tile([P, 512], F32, tag="mm")
                                for kc in range(KC2):
                                    nc.tensor.matmul(
                                        pt[:, :256], hT[:, kc, t * P:(t + 1) * P],
                                        wo_b[:, kc, :],
                                        start=(kc == 0), stop=(kc == KC2 - 1),
                                    )
                                ot = osp.tile([P, 512], F32, tag="ot")
                                if gates["bout"]:
                                    nc.vector.tensor_tensor(
                                        ot[:, :256], pt[:, :256],
                                        bout_sb[:, 512:768], OP.add)
                                    nc.vector.tensor_tensor(
                                        ot[:, :256], ot[:, :256],
                                        x1all[:, t, 512:768], OP.add)
                                else:
                                    nc.vector.tensor_tensor(
                                        ot[:, :256], pt[:, :256],
                                        x1all[:, t, 512:768], OP.add)
                                nc.sync.dma_start(outv[:, t, 512:768],
                                                  ot[:, :256])

    nc.finalize()
    return nc


def kernel(**inputs):
    global _last_results

    mm_dt_name = os.environ.get("KERNEL_MM_DT", "bf16")

    def arr(name):
        return np.ascontiguousarray(np.asarray(inputs[name], dtype=np.float32))

    x = arr("x")                       # [8, 1024, 768]
    g1 = arr("ln1_scale")
    b1 = arr("ln1_bias")
    w_qkv = arr("w_qkv")               # [768, 2304]
    b_qkv = arr("b_qkv")
    w_proj = arr("w_proj")
    b_proj = arr("b_proj")
    g2 = arr("ln2_scale")
    b2 = arr("ln2_bias")
    w_fc = arr("w_fc")
    b_fc = arr("b_fc")
    w_out = arr("w_out")
    b_out = arr("b_out")

    qscale = np.float32(1.0 / np.sqrt(D))

    w3 = w_qkv.reshape(E, H, 3, D)
    qw = (w3[:, :, 0, :].reshape(E, E) * qscale)
    kw = w3[:, :, 1, :].reshape(E, E)
    vw = w3[:, :, 2, :].reshape(E, E)
    wqk = np.ascontiguousarray(
        np.concatenate([qw, kw], axis=1) * g1[:, None]).astype(np.float32)
    wv = np.ascontiguousarray(vw * g1[:, None]).astype(np.float32)

    bq3 = (b1 @ w_qkv + b_qkv).reshape(H, 3, D)
    bqk = np.concatenate(
        [bq3[:, 0, :].reshape(E) * qscale, bq3[:, 1, :].reshape(E)]).astype(np.float32)
    bv = np.ascontiguousarray(bq3[:, 2, :].reshape(E)).astype(np.float32)

    wfc_p = np.ascontiguousarray(
        w_fc * g2[:, None] * np.float32(SELU_LAMBDA)).astype(np.float32)
    bfc_eff = (b2 @ w_fc + b_fc).astype(np.float32)
    bfce = (bfc_eff + np.float32(np.log(SELU_LA))).astype(np.float32)
    bfcl = (bfc_eff * np.float32(SELU_LAMBDA)).astype(np.float32)

    gates = {
        "bqk": bool(np.any(bqk != 0)),
        "bv": bool(np.any(bv != 0)),
        "bproj": bool(np.any(b_proj != 0)),
        "bfc": bool(np.any(bfc_eff != 0)),
        "bout": bool(np.any(b_out != 0)),
    }

    key = (tuple(sorted(gates.items())), mm_dt_name)
    if key not in _build_cache:
        _build_cache[key] = _build(gates, mm_dt_name)
    nc = _build_cache[key]

    wdt = np.float32 if mm_dt_name == "f32r" else ml_dtypes.bfloat16

    def wcast(a):
        return np.ascontiguousarray(a.astype(wdt))

    base = {
        "wqk": wcast(wqk), "wv": wcast(wv),
        "wproj": wcast(w_proj),
        "wfc": wcast(wfc_p),
        "wout": wcast(w_out),
    }
    if gates["bqk"]:
        base["bqk"] = bqk
    if gates["bv"]:
        base["bv"] = bv
    if gates["bproj"]:
        base["bproj"] = np.ascontiguousarray(b_proj)
    if gates["bfc"]:
        base["bfce"] = bfce
        base["bfcl"] = bfcl
    if gates["bout"]:
        base["bout"] = np.ascontiguousarray(b_out)

    in_maps = [dict(base, x=np.ascontiguousarray(x[c])) for c in range(NCORES)]
    res = bass_utils.run_bass_kernel_spmd(nc, in_maps, core_ids=list(range(NCORES)))
    _last_results = res
    out = np.stack([res.results[c]["out"] for c in range(NCORES)], axis=0)
    return out.astype(np.float32)


# revision 58
# speedup vs baseline: 1.1802x; 1.0110x over previous
"""Trainium2 Bass kernel for nn_AttentionBlock_68624987455817.

Pre-LN causal self-attention block + MLP (B=8, L=1024, E=768, H=12, D=64).

Sharding: data-parallel over batch B=8 across the 8 NeuronCores (one batch
element per core, weights replicated, no collectives). Each core runs the
full block on its [1024, 768] slice.

Optimized from the first working kernel (391us) to ~335us. The changes that
mattered, in order of impact:
  - all transposes on the PE (identity matmul) — the DMA-transpose path left
    the PE idle ~100us across the LN1 prologue and the LN2 mid-kernel valley.
  - per-tile LN stats (bn_stats/bn_aggr, stats emitted 3 tiles ahead) so each
    token tile's stats -> apply -> transpose -> matmul chain pipelines
    instead of barriering on all 8 tiles; LN1 fused with the V matmuls.
  - x1 (attention residual) stays in SBUF; the DRAM store/reload roundtrip
    and the separate LN2 pass are gone. proj -> LN2 -> transpose is
    software-pipelined by one tile so the PE never waits on the DVE chain.
  - ph3 software pipeline (see below): the exp/mask/copy chains of pair c
    complete under pair c+1's qk matmul stream; PV(q1) accumulation is split
    around the second S^T batch so it never waits on fresh exps.  Without
    this the recurring sub-us PE stalls also re-trigger the HAM clock gate
    (PE drops to 1.2 GHz) — the stall cost roughly doubles itself.
  - matmul operand dtype defaults to bf16 (1 cyc/col + FWL weight loads);
    rel err ~3.2e-3 vs the 2e-2 gate.  Counterintuitively bf16 alone was
    NOT faster than f32r (f32r also streams ~1 col/cycle warm); the wins
    came from the scheduling changes above.
  - whole wfc prefetched during ph4, wout passes A+B during ph5, wv/wproj
    during ph0; x tiles 0-1 DMA'd before any setup; ACT function tables
    (rsqrt/exp) preloaded off the critical path (a table switch is 1.3us).
  - engine placement tuned: psum->SBUF copies split across Vector/Scalar,
    masks+normalize on Vector, partition-broadcast on GpSimd.  GpSimd is
    ONLY safe for ops whose consumers are far away (its strict FIFO +
    per-op library reload otherwise serialize the whole pipeline — moving
    masks or LN applies there cost 2x overall).

Per-core dataflow (activations feature-major through the matmuls):
  ph0+2 per token tile t: LN1 stats (bn_stats) -> z tile -> PE transpose
        into z1T -> V matmuls (ones column per head makes P@V emit softmax
        row-sums at psum row 64).
  ph3   per head pair (2c, 2c+1), cycle: S^T(kt0-3) -> qk(c+1) -> PV(q0) ->
        PV(q1) over kt0-3 -> S^T(kt4-7) -> PV(q1) tail -> normalize.
        S^T = k_h^T q_h in [P, L] psums (seg-major emission so the two
        parities' disjoint PE row groups pack); exp -> P^T (masked);
        [O^T; sums] = Vaug^T P^T; normalize via fast reciprocal + gpsimd
        partition broadcast.
  ph4   per tile: x1 = O @ wproj + x (SBUF-resident); LN2 stats -> z2 ->
        PE transpose into z2T (pipelined one tile behind the proj matmuls).
  ph5   hT = selu(wfc^T @ z2T)  (wfc pre-scaled by selu lambda)
  ph6   out = h @ wout + x1     (two column passes; wout prefetched in ph5)

LN scales fold into the following weight matrices host-side; LN biases and
all linear biases fold into per-feature biases only materialized on-chip
when nonzero (all zero for this problem's inputs).
"""
import os
import sys

sys.path.insert(0, "/opt/trn_rl_repo")

import numpy as np
import ml_dtypes

import concourse.bass as bass
from concourse import bacc
import concourse.mybir as mybir
from concourse.tile import TileContext
from concourse import bass_utils
from concourse.masks import make_identity

F32 = mybir.dt.float32
F32R = mybir.dt.float32r
BF16 = mybir.dt.bfloat16
AF = mybir.ActivationFunctionType
OP = mybir.AluOpType
AX = mybir.AxisListType

P = 128
L = 1024
E = 768
H = 12
D = 64
DA = D + 1           # V columns + ones column (row-sum trick)
EC = E // P          # 6 feature chunks
LT = L // P          # 8 token tiles
QC = L // 512        # 2 query chunks
KC2 = 4 * E // P     # 24 chunks of the MLP hidden dim
NCORES = 8

SELU_LAMBDA = 1.0507009873554805
SELU_ALPHA = 1.6732632423543772
SELU_LA = SELU_LAMBDA * SELU_ALPHA
LN_EPS = 1e-6

_last_results = None
_build_cache = {}


def _build(gates, mm_dt_name):
    MDT = {"f32r": F32R, "bf16": BF16}[mm_dt_name]
    PSDT = MDT  # transpose output dtype must match its input dtype

    nc = bacc.Bacc("TRN2", target_bir_lowering=False)

    x_d = nc.dram_tensor("x", [L, E], F32, kind="ExternalInput")
    wqk_d = nc.dram_tensor("wqk", [E, 2 * E], MDT, kind="ExternalInput")
    wv_d = nc.dram_tensor("wv", [E, E], MDT, kind="ExternalInput")
    wproj_d = nc.dram_tensor("wproj", [E, E], MDT, kind="ExternalInput")
    wfc_d = nc.dram_tensor("wfc", [E, 4 * E], MDT, kind="ExternalInput")
    wout_d = nc.dram_tensor("wout", [4 * E, E], MDT, kind="ExternalInput")
    out_d = nc.dram_tensor("out", [L, E], F32, kind="ExternalOutput")

    bqk_d = bv_d = bproj_d = bfce_d = bfcl_d = bout_d = None
    if gates["bqk"]:
        bqk_d = nc.dram_tensor("bqk", [2 * E], F32, kind="ExternalInput")
    if gates["bv"]:
        bv_d = nc.dram_tensor("bv", [E], F32, kind="ExternalInput")
    if gates["bproj"]:
        bproj_d = nc.dram_tensor("bproj", [E], F32, kind="ExternalInput")
    if gates["bfc"]:
        bfce_d = nc.dram_tensor("bfce", [4 * E], F32, kind="ExternalInput")
        bfcl_d = nc.dram_tensor("bfcl", [4 * E], F32, kind="ExternalInput")
    if gates["bout"]:
        bout_d = nc.dram_tensor("bout", [E], F32, kind="ExternalInput")

    xv = x_d.rearrange("(t p) e -> p t e", p=P)            # [128, 8, 768]
    wqkv = wqk_d.rearrange("(c p) m -> p c m", p=P)        # [128, 6, 1536]
    wvv = wv_d.rearrange("(c p) m -> p c m", p=P)          # [128, 6, 768]
    wprojv = wproj_d.rearrange("(c p) m -> p c m", p=P)    # [128, 6, 768]
    wfcv = wfc_d.rearrange("(c p) m -> p c m", p=P)        # [128, 6, 3072]
    woutv = wout_d.rearrange("(c p) m -> p c m", p=P)      # [128, 24, 768]
    outv = out_d.rearrange("(t p) e -> p t e", p=P)

    with TileContext(nc) as tc:
        with tc.tile_pool(name="pers", bufs=1) as pers:
            xall = pers.tile([P, LT, E], F32)    # x tiles, live ph0 -> ph4
            x1all = pers.tile([P, LT, E], F32)   # x1 tiles, live ph4 -> ph6
            # x tiles 0-1 gate the whole LN1 chain: issue their DMAs before
            # any setup so the transfer overlaps the constant initialization.
            for t in range(2):
                nc.sync.dma_start(xall[:, t, :], xv[:, t, :])
            # preload the rsqrt ACT table while the DMAs run so the first LN
            # chain doesn't eat the 1.3us table load; the exp table is
            # preloaded at ph3 entry (ACT holds one table at a time).
            tabs = pers.tile([P, 1], F32)
            nc.vector.memset(tabs[:], 0.5)
            nc.scalar.activation(tabs[:], tabs[:], AF.Abs_reciprocal_sqrt)

            # mask_tri[p, f] = 1.0 if f >= p else 0.0 (keep where k <= q).
            # Built in f32 (f32r memset/affine_select fail walrus codegen).
            mask_f32 = pers.tile([P, P], F32)
            nc.gpsimd.memset(mask_f32[:], 0.0)
            nc.gpsimd.affine_select(
                out=mask_f32[:], in_=mask_f32[:],
                compare_op=OP.is_ge, fill=1.0, base=-1,
                pattern=[[-1, P]], channel_multiplier=1,
            )
            if MDT == F32R:
                mask_tri = mask_f32[:].bitcast(F32R)
            else:
                mask_b = pers.tile([P, P], BF16)
                nc.vector.tensor_copy(mask_b[:], mask_f32[:])
                mask_tri = mask_b[:]
            ones_f32 = pers.tile([P, LT * H], F32)
            nc.vector.memset(ones_f32[:], 1.0)
            eps_b = pers.tile([P, 1], F32)
            nc.vector.memset(eps_b[:], LN_EPS)
            lnla_b = pers.tile([P, 1], F32)
            nc.vector.memset(lnla_b[:], float(np.log(SELU_LA)))

            ident = pers.tile([P, P], F32)
            make_identity(nc, ident)
            ident_m = pers.tile([P, P], MDT)
            nc.vector.tensor_copy(ident_m[:], ident[:])

            bqk_sb = bv_sb = bproj_sb = bfce_sb = bfcl_sb = bout_sb = None
            if gates["bqk"]:
                bqk_sb = pers.tile([P, 2 * EC], F32)
                nc.sync.dma_start(bqk_sb[:], bqk_d.rearrange("(c p) -> p c", p=P))
            if gates["bv"]:
                bv_sb = pers.tile([P, E], F32)
                nc.sync.dma_start(bv_sb[:], bv_d.to_broadcast((P, E)))
            if gates["bproj"]:
                bproj_sb = pers.tile([P, E], F32)
                nc.sync.dma_start(bproj_sb[:], bproj_d.to_broadcast((P, E)))
            if gates["bfc"]:
                bfce_sb = pers.tile([P, KC2], F32)
                nc.sync.dma_start(bfce_sb[:], bfce_d.rearrange("(c p) -> p c", p=P))
                bfcl_sb = pers.tile([P, KC2], F32)
                nc.sync.dma_start(bfcl_sb[:], bfcl_d.rearrange("(c p) -> p c", p=P))
            if gates["bout"]:
                bout_sb = pers.tile([P, E], F32)
                nc.sync.dma_start(bout_sb[:], bout_d.to_broadcast((P, E)))

            def transpose_block(dstT, src_tile, t, pspool):
                """dstT[:, c, t*P:(t+1)*P] = src_tile[:, c*P:(c+1)*P].T"""
                for c in range(EC):
                    pt = pspool.tile([P, P], PSDT, tag="tr")
                    nc.tensor.transpose(pt[:], src_tile[:, c * P:(c + 1) * P],
                                        ident_m[:])
                    nc.scalar.copy(out=dstT[:, c, t * P:(t + 1) * P],
                                   in_=pt[:])

            def ln_tile(src, stpool, tag):
                """bn_stats mean/var over the free axis + rsqrt(var+eps)."""
                bnst = stpool.tile([P, 2, 6], F32, tag=f"bn{tag}", name="bnst")
                xg = src.rearrange("p (n f) -> p n f", f=384)
                for g in range(2):
                    nc.vector.bn_stats(bnst[:, g, :], xg[:, g, :])
                mv = stpool.tile([P, 2], F32, tag=f"mv{tag}", name="mv")
                nc.vector.bn_aggr(mv[:], bnst[:])
                rt = stpool.tile([P, 1], F32, tag=f"rt{tag}", name="rt")
                nc.scalar.activation(rt[:], mv[:, 1:2], AF.Abs_reciprocal_sqrt,
                                     bias=eps_b[:])
                return mv, rt

            with tc.tile_pool(name="fm", bufs=1) as fmp:
                z1T = fmp.tile([P, EC, L], MDT, tag="fm", name="z1T")
                with tc.tile_pool(name="wfcp", bufs=1) as wfcp:
                    wfc_sb = wfcp.tile([P, EC, 4 * E], MDT)
                    with (
                        tc.tile_pool(name="otp", bufs=1) as otp,
                        tc.tile_pool(name="vp", bufs=1) as vpool,
                        tc.tile_pool(name="wpp", bufs=1) as wpp,
                    ):
                        OT = otp.tile([P, EC, L], MDT)
                        v_aug = vpool.tile([P, LT, H, DA], MDT)
                        wproj_sb = wpp.tile([P, EC, E], MDT)

                        # ---- ph0+ph2 fused: per-tile LN1 -> z1T -> V ----
                        with (
                            tc.tile_pool(name="wvp", bufs=1) as wvp,
                            tc.tile_pool(name="zp", bufs=2) as zp,
                            tc.tile_pool(name="stp", bufs=4) as stp,
                            tc.tile_pool(name="ps0", bufs=4, space="PSUM") as ps0,
                            tc.tile_pool(name="ps2", bufs=4, space="PSUM") as ps2,
                        ):
                            wv_sb = wvp.tile([P, EC, E], MDT)
                            nc.vector.tensor_copy(
                                v_aug[:, :, :, D:DA],
                                ones_f32[:].rearrange("p (t h o) -> p t h o",
                                                      h=H, o=1))
                            # x tiles 0-1 already in flight (issued at pers
                            # setup); wv next (needed by V at ~7us), then the
                            # rest of x, then wproj.
                            nc.sync.dma_start(wv_sb[:], wvv[:])
                            for t in range(2, LT):
                                nc.sync.dma_start(xall[:, t, :], xv[:, t, :])
                            nc.sync.dma_start(wproj_sb[:], wprojv[:])
                            # LN1 stats run 3 tiles ahead so the ACT rsqrt is
                            # never queued behind transpose copies.
                            stats1 = {}
                            for t in range(3):
                                stats1[t] = ln_tile(xall[:, t, :], stp, "1")
                            for t in range(LT):
                                if t + 3 < LT:
                                    stats1[t + 3] = ln_tile(
                                        xall[:, t + 3, :], stp, "1")
                                mv, rt = stats1.pop(t)
                                zt = zp.tile([P, E], MDT, tag="z")
                                nc.vector.tensor_scalar(
                                    zt[:], xall[:, t, :], mv[:, 0:1], rt[:],
                                    OP.subtract, OP.mult,
                                )
                                transpose_block(z1T, zt, t, ps0)
                                # V matmuls for tile t
                                for (c0, cw) in ((0, 512), (512, 256)):
                                    pt2 = ps2.tile([P, 512], F32, tag="mm")
                                    for kc in range(EC):
                                        nc.tensor.matmul(
                                            pt2[:, :cw],
                                            z1T[:, kc, t * P:(t + 1) * P],
                                            wv_sb[:, kc, c0:c0 + cw],
                                            start=(kc == 0), stop=(kc == EC - 1),
                                        )
                                    h0 = c0 // D
                                    nh = cw // D
                                    dst = v_aug[:, t, h0:h0 + nh, 0:D]
                                    if gates["bv"]:
                                        nc.vector.tensor_tensor(
                                            dst,
                                            pt2[:, :cw].rearrange(
                                                "p (h d) -> p h d", d=D),
                                            bv_sb[:, c0:c0 + cw].rearrange(
                                                "p (h d) -> p h d", d=D),
                                            OP.add)
                                    else:
                                        nc.vector.tensor_copy(
                                            out=dst,
                                            in_=pt2[:, :cw].rearrange(
                                                "p (h d) -> p h d", d=D))

                        # ---- ph3: per head pair qk + attention ----
                        # Cycle emission: S^T(all kt) -> PV(q0) -> PV(q1) ->
                        # norms -> qk(c+1).  The qk stream at the cycle end is
                        # 5us of independent PE work during which the next
                        # cycle's psum->SBUF copies and this cycle's norm
                        # chains drain, so the PE re-enters S^T with all
                        # dependencies met.  qk borrows the [P, L] S^T psum
                        # tiles (both lc halves in one tile); PV has its own
                        # 4-bank pool so no intra-cycle WAR aliasing exists.
                        with (
                            tc.tile_pool(name="qkpp", bufs=2) as qkpp,
                            tc.tile_pool(name="wqks", bufs=2) as wqs,
                            tc.tile_pool(name="ptp", bufs=1) as ptp,
                            tc.tile_pool(name="recp", bufs=2) as recp,
                            tc.tile_pool(name="ps3", bufs=4, space="PSUM") as ps3,
                            tc.tile_pool(name="ps3s", bufs=2, space="PSUM") as ps3s,
                        ):
                            # preload the Exp table during qk(0) so the first
                            # real exp doesn't eat a 1.3us table load
                            nc.scalar.activation(tabs[:], tabs[:], AF.Exp)

                            def emit_qk(c):
                                qk_pair = qkpp.tile([P, 2, L], MDT, tag="qkpair",
                                                    name=f"qkp{c}")
                                for i, oc in enumerate((c, EC + c)):
                                    wt = wqs.tile([P, EC, P], MDT, tag="wqk",
                                                  name=f"wqk{c}_{i}")
                                    nc.sync.dma_start(
                                        wt[:], wqkv[:, :, oc * P:(oc + 1) * P])
                                    psums = [ps3.tile([P, 512], F32, tag="mm",
                                                      name=f"qkps{lc}")
                                             for lc in range(QC)]
                                    for kc in range(EC):
                                        for lc in range(QC):
                                            nc.tensor.matmul(
                                                psums[lc][:], wt[:, kc, :],
                                                z1T[:, kc, lc * 512:(lc + 1) * 512],
                                                start=(kc == 0),
                                                stop=(kc == EC - 1),
                                            )
                                    for lc in range(QC):
                                        dst = qk_pair[:, i, lc * 512:(lc + 1) * 512]
                                        if gates["bqk"]:
                                            nc.scalar.activation(
                                                dst, psums[lc][:], AF.Identity,
                                                bias=bqk_sb[:, oc:oc + 1])
                                        else:
                                            # keep these on DVE: a cast on
                                            # the ACT queue sits between exps
                                            # and delays the psum-ring WARs
                                            nc.vector.tensor_copy(
                                                out=dst, in_=psums[lc][:])
                                return qk_pair

                            def emit_st(qk_pair, PTs, kt):
                                # S^T for key tile kt: full row s0:L in one
                                # [P, L] psum tile so same-lhsT matmuls go
                                # back-to-back (pipelined drains), exp split
                                # per query half into PTs[qc][par].
                                s0 = kt * P
                                segs = ([(s0, 512), (512, L)] if s0 < 512
                                        else [(s0, L)])
                                psss = [ps3s.tile([P, L], F32, tag="st",
                                                  name=f"pss{par}")
                                        for par in range(2)]
                                # seg-major emission: the two parities of one
                                # segment are FIFO-adjacent, so their disjoint
                                # PE row groups run concurrently (row packing);
                                # par-major order lets par0's second segment
                                # (same rows as its first) block par1's start.
                                for (a, b) in segs:
                                    for par in range(2):
                                        rows = slice(par * D, par * D + D)
                                        nc.tensor.matmul(
                                            psss[par][:, a:b],
                                            qk_pair[rows, 1, s0:s0 + P],
                                            qk_pair[rows, 0, a:b],
                                            start=True, stop=True)
                                for par in range(2):
                                    pss = psss[par]
                                    if s0 < 512:
                                        nc.scalar.activation(
                                            PTs[0][par][:, kt, s0:512],
                                            pss[:, s0:512], AF.Exp)
                                        nc.scalar.activation(
                                            PTs[1][par][:, kt, 0:512],
                                            pss[:, 512:L], AF.Exp)
                                        nc.vector.tensor_tensor(
                                            PTs[0][par][:, kt, s0:s0 + P],
                                            PTs[0][par][:, kt, s0:s0 + P],
                                            mask_tri, OP.mult)
                                    else:
                                        nc.scalar.activation(
                                            PTs[1][par][:, kt, s0 - 512:512],
                                            pss[:, s0:L], AF.Exp)
                                        nc.vector.tensor_tensor(
                                            PTs[1][par][:, kt,
                                                        s0 - 512:s0 - 512 + P],
                                            PTs[1][par][:, kt,
                                                        s0 - 512:s0 - 512 + P],
                                            mask_tri, OP.mult)

                            def emit_pv_part(c, qc, PTx, psos, kts, first, last):
                                # P@V accumulation sub-group over key tiles
                                # `kts`; the group may be split around other
                                # matmuls (start only on `first`, stop on
                                # `last`).
                                q0 = qc * 512
                                for par in range(2):
                                    h = 2 * c + par
                                    pt_buf = PTx[par]
                                    pso = psos[par]
                                    for idx, j in enumerate(kts):
                                        a = max(j * P, q0)
                                        nc.tensor.matmul(
                                            pso[0:DA, a - q0:512],
                                            v_aug[:, j, h, :],
                                            pt_buf[:, j, a - q0:512],
                                            start=(first and idx == 0),
                                            stop=(last and idx == len(kts) - 1))

                            def emit_pv_norm(c, qc, psos):
                                # psum row 64 carries softmax row-sums (the
                                # ones column of v_aug).  Copy O^T and the
                                # sums row straight out of PSUM so the PV
                                # bank frees after the copies (the bank's
                                # next writer — the coming cycle's qk
                                # matmuls — was measured stalling 1.5us/pair
                                # waiting on the old copy->recip->broadcast->
                                # mult chain); the normalize then runs from
                                # SBUF off the bank-release path.  The
                                # reciprocal input must sit at partition 0
                                # (the custom-DVE seed NaNs on a base-64 row).
                                q0 = qc * 512
                                for par in range(2):
                                    pso = psos[par]
                                    o_rows = slice(par * D, par * D + D)
                                    osb = recp.tile([P, 512], F32, tag="ob")
                                    if qc == 0:
                                        # ACT's queue is exp-free right here
                                        nc.scalar.copy(out=osb[0:D, :],
                                                       in_=pso[0:D, :])
                                    else:
                                        nc.vector.tensor_copy(osb[0:D, :],
                                                              pso[0:D, :])
                                    srow = recp.tile([P, 512], F32, tag="sr")
                                    nc.vector.tensor_copy(srow[0:1, :],
                                                          pso[D:DA, :])
                                    rec = recp.tile([P, 512], F32, tag="rc")
                                    nc.vector.reciprocal_approx_fast(
                                        rec[0:1, :], srow[0:1, :])
                                    recb = recp.tile([P, 512], F32, tag="rb")
                                    nc.gpsimd.partition_broadcast(
                                        recb[0:D, :], rec[0:1, :])
                                    nc.vector.tensor_tensor(
                                        OT[o_rows, c, q0:q0 + 512],
                                        osb[0:D, :], recb[0:D, :], OP.mult,
                                    )

                            # Per-pair cycle, software-pipelined so the PE
                            # never waits on the exp/mask/copy chains:
                            #   S^T(kt0-3) -> qk(c+1) [5us of independent PE
                            #   work while exps+masks drain] -> PV(q0) ->
                            #   PV(q1) over kt0-3 -> S^T(kt4-7) -> PV(q1)
                            #   tail.  The PV(q1) accumulation group stays
                            #   open across the second S^T batch.
                            qkp_cur = emit_qk(0)
                            for c in range(EC):  # head pair (2c, 2c+1)
                                PTs = [[ptp.tile([P, LT, 512], MDT,
                                                 tag=f"pt{qc}{par}",
                                                 name=f"pt{qc}{par}")
                                        for par in range(2)]
                                       for qc in range(QC)]
                                for kt in range(4):
                                    emit_st(qkp_cur, PTs, kt)
                                qkp_next = (emit_qk(c + 1) if c + 1 < EC
                                            else None)
                                psos0 = [ps3.tile([P, 512], F32, tag="mm",
                                                  name=f"pv0{par}")
                                         for par in range(2)]
                                psos1 = [ps3.tile([P, 512], F32, tag="mm",
                                                  name=f"pv1{par}")
                                         for par in range(2)]
                                emit_pv_part(c, 0, PTs[0], psos0,
                                             range(4), True, True)
                                emit_pv_part(c, 1, PTs[1], psos1,
                                             range(4), True, False)
                                for kt in range(4, LT):
                                    emit_st(qkp_cur, PTs, kt)
                                emit_pv_norm(c, 0, psos0)
                                emit_pv_part(c, 1, PTs[1], psos1,
                                             range(4, LT), False, True)
                                emit_pv_norm(c, 1, psos1)
                                qkp_cur = qkp_next
                            # reload the rsqrt table while the PE finishes the
                            # last pair, so ph4's first LN2 chain doesn't wait
                            # the 1.3us load inside the phase-boundary gap
                            nc.scalar.activation(tabs[:], tabs[:],
                                                 AF.Abs_reciprocal_sqrt)

                        # ---- ph4: proj + residual + LN2 + transpose ----
                        z2T = fmp.tile([P, EC, L], MDT, tag="fm", name="z2T")
                        with (
                            tc.tile_pool(name="zp2", bufs=2) as zp2,
                            tc.tile_pool(name="stp2", bufs=2) as stp2,
                            tc.tile_pool(name="ps4", bufs=4, space="PSUM") as ps4,
                            tc.tile_pool(name="ps45", bufs=4, space="PSUM") as ps45,
                        ):
                            # prefetch whole wfc during ph4 (8 chunks so ph5's
                            # first oc doesn't wait on one giant DMA)
                            for j in range(8):
                                nc.sync.dma_start(
                                    wfc_sb[:, :, j * 384:(j + 1) * 384],
                                    wfcv[:, :, j * 384:(j + 1) * 384])
                            prev_z2t = None
                            for t in range(LT):
                                for (c0, cw) in ((0, 512), (512, 256)):
                                    pt = ps4.tile([P, 512], F32, tag="mm")
                                    for kc in range(EC):
                                        nc.tensor.matmul(
                                            pt[:, :cw],
                                            OT[:, kc, t * P:(t + 1) * P],
                                            wproj_sb[:, kc, c0:c0 + cw],
                                            start=(kc == 0), stop=(kc == EC - 1),
                                        )
                                    dst = x1all[:, t, c0:c0 + cw]
                                    if gates["bproj"]:
                                        nc.vector.tensor_tensor(
                                            dst, pt[:, :cw],
                                            bproj_sb[:, c0:c0 + cw], OP.add)
                                        nc.vector.tensor_tensor(
                                            dst, dst, xall[:, t, c0:c0 + cw],
                                            OP.add)
                                    else:
                                        nc.vector.tensor_tensor(
                                            dst, pt[:, :cw],
                                            xall[:, t, c0:c0 + cw], OP.add)
                                mv2, rt2 = ln_tile(x1all[:, t, :], stp2, "2")
                                z2t = zp2.tile([P, E], MDT, tag="z2")
                                nc.vector.tensor_scalar(
                                    z2t[:], x1all[:, t, :], mv2[:, 0:1], rt2[:],
                                    OP.subtract, OP.mult,
                                )
                                # transposes lag one tile so the PE never
                                # waits on the DVE stats/apply chain
                                if prev_z2t is not None:
                                    transpose_block(z2T, prev_z2t, t - 1, ps45)
                                prev_z2t = z2t
                            transpose_block(z2T, prev_z2t, LT - 1, ps45)
                            # preload the exp table for ph5's selu during the
                            # last transposes / first fc matmuls
                            nc.scalar.activation(tabs[:], tabs[:], AF.Exp)

                    # ---- ph5: fc + selu -> hT ----
                    with (
                        tc.tile_pool(name="htp", bufs=1) as htp,
                        tc.tile_pool(name="wop", bufs=1) as wop,
                    ):
                        hT = htp.tile([P, KC2, L], MDT)
                        wo_a = wop.tile([P, KC2, 512], MDT)
                        nc.sync.dma_start(wo_a[:], woutv[:, :, 0:512])
                        wo_b = wop.tile([P, KC2, 256], MDT)
                        nc.sync.dma_start(wo_b[:], woutv[:, :, 512:768])
                        with (
                            tc.tile_pool(name="selu", bufs=2) as slp,
                            # one psum ring spans ph5 AND ph6 so the phase
                            # boundary has no pool-reuse WAR gap
                            tc.tile_pool(name="ps5", bufs=4, space="PSUM") as ps5,
                        ):
                            for oc in range(KC2):
                                for lc in range(QC):
                                    pt = ps5.tile([P, 512], F32, tag="mm")
                                    for kc in range(EC):
                                        nc.tensor.matmul(
                                            pt[:],
                                            wfc_sb[:, kc, oc * P:(oc + 1) * P],
                                            z2T[:, kc, lc * 512:(lc + 1) * 512],
                                            start=(kc == 0), stop=(kc == EC - 1),
                                        )
                                    pe_t = slp.tile([P, 512], F32, tag="pe")
                                    bias = (bfce_sb[:, oc:oc + 1] if gates["bfc"]
                                            else lnla_b[:])
                                    nc.scalar.activation(pe_t[:], pt[:], AF.Exp,
                                                         bias=bias,
                                                         scale=1.0 / SELU_LAMBDA)
                                    a_t = slp.tile([P, 512], F32, tag="at")
                                    nc.vector.tensor_scalar(
                                        a_t[:], pe_t[:], SELU_LA, SELU_LA,
                                        OP.min, OP.subtract)
                                    dst = hT[:, oc, lc * 512:(lc + 1) * 512]
                                    if gates["bfc"]:
                                        rl = slp.tile([P, 512], F32, tag="rl")
                                        nc.vector.tensor_scalar(
                                            rl[:], pt[:], bfcl_sb[:, oc:oc + 1],
                                            0.0, OP.add, OP.max)
                                        nc.vector.tensor_tensor(dst, rl[:],
                                                                a_t[:], OP.add)
                                    else:
                                        nc.vector.scalar_tensor_tensor(
                                            dst, pt[:], 0.0, a_t[:],
                                            OP.max, OP.add)

                            # ---- ph6: out = h @ wout + x1 (two passes) ----
                            with tc.tile_pool(name="osp", bufs=3) as osp:
                                ps6 = ps5
                            for t in range(LT):
                                pt = ps6.tile([P, 512], F32, tag="mm")
                                for kc in range(KC2):
                                    nc.tensor.matmul(
                                        pt[:], hT[:, kc, t * P:(t + 1) * P],
                                        wo_a[:, kc, :],
                                        start=(kc == 0), stop=(kc == KC2 - 1),
                                    )
                                ot = osp.tile([P, 512], F32, tag="ot")
                                if gates["bout"]:
                                    nc.vector.tensor_tensor(
                                        ot[:], pt[:], bout_sb[:, 0:512], OP.add)
                                    nc.vector.tensor_tensor(
                                        ot[:], ot[:], x1all[:, t, 0:512], OP.add)
                                else:
                                    nc.vector.tensor_tensor(
                                        ot[:], pt[:], x1all[:, t, 0:512], OP.add)
                                nc.sync.dma_start(outv[:, t, 0:512], ot[:])

                            for t in range(LT):
                                pt = ps6.tile([P, 512], F32, tag="mm")
                                for kc in range(KC2):
                                    nc.tensor.matmul(
                                        pt[:, :256], hT[:, kc, t * P:(t + 1) * P],
                                        wo_b[:, kc, :],
                                        start=(kc == 0), stop=(kc == KC2 - 1),
                                    )
                                ot = osp.tile([P, 512], F32, tag="ot")
                                if gates["bout"]:
                                    nc.vector.tensor_tensor(
                                        ot[:, :256], pt[:, :256],
                                        bout_sb[:, 512:768], OP.add)
                                    nc.vector.tensor_tensor(
                                        ot[:, :256], ot[:, :256],
                                        x1all[:, t, 512:768], OP.add)
                                else:
                                    nc.vector.tensor_tensor(
                                        ot[:, :256], pt[:, :256],
                                        x1all[:, t, 512:768], OP.add)
                                nc.sync.dma_start(outv[:, t, 512:768],
                                                  ot[:, :256])

    nc.finalize()
    return nc


def kernel(**inputs):
    global _last_results

    mm_dt_name = os.environ.get("KERNEL_MM_DT", "bf16")

    def arr(name):
        return np.ascontiguousarray(np.asarray(inputs[name], dtype=np.float32))

    x = arr("x")                       # [8, 1024, 768]
    g1 = arr("ln1_scale")
    b1 = arr("ln1_bias")
    w_qkv = arr("w_qkv")               # [768, 2304]
    b_qkv = arr("b_qkv")
    w_proj = arr("w_proj")
    b_proj = arr("b_proj")
    g2 = arr("ln2_scale")
    b2 = arr("ln2_bias")
    w_fc = arr("w_fc")
    b_fc = arr("b_fc")
    w_out = arr("w_out")
    b_out = arr("b_out")

    qscale = np.float32(1.0 / np.sqrt(D))

    w3 = w_qkv.reshape(E, H, 3, D)
    qw = (w3[:, :, 0, :].reshape(E, E) * qscale)
    kw = w3[:, :, 1, :].reshape(E, E)
    vw = w3[:, :, 2, :].reshape(E, E)
    wqk = np.ascontiguousarray(
        np.concatenate([qw, kw], axis=1) * g1[:, None]).astype(np.float32)
    wv = np.ascontiguousarray(vw * g1[:, None]).astype(np.float32)

    bq3 = (b1 @ w_qkv + b_qkv).reshape(H, 3, D)
    bqk = np.concatenate(
        [bq3[:, 0, :].reshape(E) * qscale, bq3[:, 1, :].reshape(E)]).astype(np.float32)
    bv = np.ascontiguousarray(bq3[:, 2, :].reshape(E)).astype(np.float32)

    wfc_p = np.ascontiguousarray(
        w_fc * g2[:, None] * np.float32(SELU_LAMBDA)).astype(np.float32)
    bfc_eff = (b2 @ w_fc + b_fc).astype(np.float32)
    bfce = (bfc_eff + np.float32(np.log(SELU_LA))).astype(np.float32)
    bfcl = (bfc_eff * np.float32(SELU_LAMBDA)).astype(np.float32)

    gates = {
        "bqk": bool(np.any(bqk != 0)),
        "bv": bool(np.any(bv != 0)),
        "bproj": bool(np.any(b_proj != 0)),
        "bfc": bool(np.any(bfc_eff != 0)),
        "bout": bool(np.any(b_out != 0)),
    }

    key = (tuple(sorted(gates.items())), mm_dt_name)
    if key not in _build_cache:
        _build_cache[key] = _build(gates, mm_dt_name)
    nc = _build_cache[key]

    wdt = np.float32 if mm_dt_name == "f32r" else ml_dtypes.bfloat16

    def wcast(a):
        return np.ascontiguousarray(a.astype(wdt))

    base = {
        "wqk": wcast(wqk), "wv": wcast(wv),
        "wproj": wcast(w_proj),
        "wfc": wcast(wfc_p),
        "wout": wcast(w_out),
    }
    if gates["bqk"]:
        base["bqk"] = bqk
    if gates["bv"]:
        base["bv"] = bv
    if gates["bproj"]:
        base["bproj"] = np.ascontiguousarray(b_proj)
    if gates["bfc"]:
        base["bfce"] = bfce
        base["bfcl"] = bfcl
    if gates["bout"]:
        base["bout"] = np.ascontiguousarray(b_out)

    in_maps = [dict(base, x=np.ascontiguousarray(x[c])) for c in range(NCORES)]
    res = bass_utils.run_bass_kernel_spmd(nc, in_maps, core_ids=list(range(NCORES)))
    _last_results = res
    out = np.stack([res.results[c]["out"] for c in range(NCORES)], axis=0)
    return out.astype(np.float32)


# revision 59
# speedup vs baseline: 1.2060x; 1.0218x over previous
"""Trainium2 Bass kernel for nn_AttentionBlock_68624987455817.

Pre-LN causal self-attention block + MLP (B=8, L=1024, E=768, H=12, D=64).

Sharding: data-parallel over batch B=8 across the 8 NeuronCores (one batch
element per core, weights replicated, no collectives). Each core runs the
full block on its [1024, 768] slice.

Optimized from the first working kernel (391us) to ~335us. The changes that
mattered, in order of impact:
  - all transposes on the PE (identity matmul) — the DMA-transpose path left
    the PE idle ~100us across the LN1 prologue and the LN2 mid-kernel valley.
  - per-tile LN stats (bn_stats/bn_aggr, stats emitted 3 tiles ahead) so each
    token tile's stats -> apply -> transpose -> matmul chain pipelines
    instead of barriering on all 8 tiles; LN1 fused with the V matmuls.
  - x1 (attention residual) stays in SBUF; the DRAM store/reload roundtrip
    and the separate LN2 pass are gone. proj -> LN2 -> transpose is
    software-pipelined by one tile so the PE never waits on the DVE chain.
  - ph3 software pipeline (see below): the exp/mask/copy chains of pair c
    complete under pair c+1's qk matmul stream; PV(q1) accumulation is split
    around the second S^T batch so it never waits on fresh exps.  Without
    this the recurring sub-us PE stalls also re-trigger the HAM clock gate
    (PE drops to 1.2 GHz) — the stall cost roughly doubles itself.
  - matmul operand dtype defaults to bf16 (1 cyc/col + FWL weight loads);
    rel err ~3.2e-3 vs the 2e-2 gate.  Counterintuitively bf16 alone was
    NOT faster than f32r (f32r also streams ~1 col/cycle warm); the wins
    came from the scheduling changes above.
  - whole wfc prefetched during ph4, wout passes A+B during ph5, wv/wproj
    during ph0; x tiles 0-1 DMA'd before any setup; ACT function tables
    (rsqrt/exp) preloaded off the critical path (a table switch is 1.3us).
  - engine placement tuned: psum->SBUF copies split across Vector/Scalar,
    masks+normalize on Vector, partition-broadcast on GpSimd.  GpSimd is
    ONLY safe for ops whose consumers are far away (its strict FIFO +
    per-op library reload otherwise serialize the whole pipeline — moving
    masks or LN applies there cost 2x overall).

Per-core dataflow (activations feature-major through the matmuls):
  ph0+2 per token tile t: LN1 stats (bn_stats) -> z tile -> PE transpose
        into z1T -> V matmuls (ones column per head makes P@V emit softmax
        row-sums at psum row 64).
  ph3   per head pair (2c, 2c+1), cycle: S^T(kt0-3) -> qk(c+1) -> PV(q0) ->
        PV(q1) over kt0-3 -> S^T(kt4-7) -> PV(q1) tail -> normalize.
        S^T = k_h^T q_h in [P, L] psums (seg-major emission so the two
        parities' disjoint PE row groups pack); exp -> P^T (masked);
        [O^T; sums] = Vaug^T P^T; normalize via fast reciprocal + gpsimd
        partition broadcast.
  ph4   per tile: x1 = O @ wproj + x (SBUF-resident); LN2 stats -> z2 ->
        PE transpose into z2T (pipelined one tile behind the proj matmuls).
  ph5   hT = selu(wfc^T @ z2T)  (wfc pre-scaled by selu lambda)
  ph6   out = h @ wout + x1     (two column passes; wout prefetched in ph5)

LN scales fold into the following weight matrices host-side; LN biases and
all linear biases fold into per-feature biases only materialized on-chip
when nonzero (all zero for this problem's inputs).
"""
import os
import sys

sys.path.insert(0, "/opt/trn_rl_repo")

import numpy as np
import ml_dtypes

import concourse.bass as bass
from concourse import bacc
import concourse.mybir as mybir
from concourse.tile import TileContext
from concourse import bass_utils
from concourse.masks import make_identity

F32 = mybir.dt.float32
F32R = mybir.dt.float32r
BF16 = mybir.dt.bfloat16
AF = mybir.ActivationFunctionType
OP = mybir.AluOpType
AX = mybir.AxisListType

P = 128
L = 1024
E = 768
H = 12
D = 64
DA = D + 1           # V columns + ones column (row-sum trick)
EC = E // P          # 6 feature chunks
LT = L // P          # 8 token tiles
QC = L // 512        # 2 query chunks
KC2 = 4 * E // P     # 24 chunks of the MLP hidden dim
NCORES = 8

SELU_LAMBDA = 1.0507009873554805
SELU_ALPHA = 1.6732632423543772
SELU_LA = SELU_LAMBDA * SELU_ALPHA
LN_EPS = 1e-6

_last_results = None
_build_cache = {}


def _build(gates, mm_dt_name):
    MDT = {"f32r": F32R, "bf16": BF16}[mm_dt_name]
    PSDT = MDT  # transpose output dtype must match its input dtype

    nc = bacc.Bacc("TRN2", target_bir_lowering=False)

    x_d = nc.dram_tensor("x", [L, E], F32, kind="ExternalInput")
    wqk_d = nc.dram_tensor("wqk", [E, 2 * E], MDT, kind="ExternalInput")
    wv_d = nc.dram_tensor("wv", [E, E], MDT, kind="ExternalInput")
    wproj_d = nc.dram_tensor("wproj", [E, E], MDT, kind="ExternalInput")
    wfc_d = nc.dram_tensor("wfc", [E, 4 * E], MDT, kind="ExternalInput")
    wout_d = nc.dram_tensor("wout", [4 * E, E], MDT, kind="ExternalInput")
    out_d = nc.dram_tensor("out", [L, E], F32, kind="ExternalOutput")

    bqk_d = bv_d = bproj_d = bfce_d = bfcl_d = bout_d = None
    if gates["bqk"]:
        bqk_d = nc.dram_tensor("bqk", [2 * E], F32, kind="ExternalInput")
    if gates["bv"]:
        bv_d = nc.dram_tensor("bv", [E], F32, kind="ExternalInput")
    if gates["bproj"]:
        bproj_d = nc.dram_tensor("bproj", [E], F32, kind="ExternalInput")
    if gates["bfc"]:
        bfce_d = nc.dram_tensor("bfce", [4 * E], F32, kind="ExternalInput")
        bfcl_d = nc.dram_tensor("bfcl", [4 * E], F32, kind="ExternalInput")
    if gates["bout"]:
        bout_d = nc.dram_tensor("bout", [E], F32, kind="ExternalInput")

    xv = x_d.rearrange("(t p) e -> p t e", p=P)            # [128, 8, 768]
    wqkv = wqk_d.rearrange("(c p) m -> p c m", p=P)        # [128, 6, 1536]
    wvv = wv_d.rearrange("(c p) m -> p c m", p=P)          # [128, 6, 768]
    wprojv = wproj_d.rearrange("(c p) m -> p c m", p=P)    # [128, 6, 768]
    wfcv = wfc_d.rearrange("(c p) m -> p c m", p=P)        # [128, 6, 3072]
    woutv = wout_d.rearrange("(c p) m -> p c m", p=P)      # [128, 24, 768]
    outv = out_d.rearrange("(t p) e -> p t e", p=P)

    with TileContext(nc) as tc:
        with tc.tile_pool(name="pers", bufs=1) as pers:
            xall = pers.tile([P, LT, E], F32)    # x tiles, live ph0 -> ph4
            x1all = pers.tile([P, LT, E], F32)   # x1 tiles, live ph4 -> ph6
            # x tiles 0-1 gate the whole LN1 chain: issue their DMAs before
            # any setup so the transfer overlaps the constant initialization.
            for t in range(2):
                nc.sync.dma_start(xall[:, t, :], xv[:, t, :])
            # preload the rsqrt ACT table while the DMAs run so the first LN
            # chain doesn't eat the 1.3us table load; the exp table is
            # preloaded at ph3 entry (ACT holds one table at a time).
            tabs = pers.tile([P, 1], F32)
            nc.vector.memset(tabs[:], 0.5)
            nc.scalar.activation(tabs[:], tabs[:], AF.Abs_reciprocal_sqrt)

            # mask_tri[p, f] = 1.0 if f >= p else 0.0 (keep where k <= q).
            # Built in f32 (f32r memset/affine_select fail walrus codegen).
            mask_f32 = pers.tile([P, P], F32)
            nc.gpsimd.memset(mask_f32[:], 0.0)
            nc.gpsimd.affine_select(
                out=mask_f32[:], in_=mask_f32[:],
                compare_op=OP.is_ge, fill=1.0, base=-1,
                pattern=[[-1, P]], channel_multiplier=1,
            )
            if MDT == F32R:
                mask_tri = mask_f32[:].bitcast(F32R)
            else:
                mask_b = pers.tile([P, P], BF16)
                nc.vector.tensor_copy(mask_b[:], mask_f32[:])
                mask_tri = mask_b[:]
            ones_f32 = pers.tile([P, LT * H], F32)
            nc.vector.memset(ones_f32[:], 1.0)
            eps_b = pers.tile([P, 1], F32)
            nc.vector.memset(eps_b[:], LN_EPS)
            lnla_b = pers.tile([P, 1], F32)
            nc.vector.memset(lnla_b[:], float(np.log(SELU_LA)))

            ident = pers.tile([P, P], F32)
            make_identity(nc, ident)
            ident_m = pers.tile([P, P], MDT)
            nc.vector.tensor_copy(ident_m[:], ident[:])

            bqk_sb = bv_sb = bproj_sb = bfce_sb = bfcl_sb = bout_sb = None
            if gates["bqk"]:
                bqk_sb = pers.tile([P, 2 * EC], F32)
                nc.sync.dma_start(bqk_sb[:], bqk_d.rearrange("(c p) -> p c", p=P))
            if gates["bv"]:
                bv_sb = pers.tile([P, E], F32)
                nc.sync.dma_start(bv_sb[:], bv_d.to_broadcast((P, E)))
            if gates["bproj"]:
                bproj_sb = pers.tile([P, E], F32)
                nc.sync.dma_start(bproj_sb[:], bproj_d.to_broadcast((P, E)))
            if gates["bfc"]:
                bfce_sb = pers.tile([P, KC2], F32)
                nc.sync.dma_start(bfce_sb[:], bfce_d.rearrange("(c p) -> p c", p=P))
                bfcl_sb = pers.tile([P, KC2], F32)
                nc.sync.dma_start(bfcl_sb[:], bfcl_d.rearrange("(c p) -> p c", p=P))
            if gates["bout"]:
                bout_sb = pers.tile([P, E], F32)
                nc.sync.dma_start(bout_sb[:], bout_d.to_broadcast((P, E)))

            def transpose_block(dstT, src_tile, t, pspool):
                """dstT[:, c, t*P:(t+1)*P] = src_tile[:, c*P:(c+1)*P].T"""
                for c in range(EC):
                    pt = pspool.tile([P, P], PSDT, tag="tr")
                    nc.tensor.transpose(pt[:], src_tile[:, c * P:(c + 1) * P],
                                        ident_m[:])
                    nc.scalar.copy(out=dstT[:, c, t * P:(t + 1) * P],
                                   in_=pt[:])

            def ln_tile(src, stpool, tag):
                """bn_stats mean/var over the free axis + rsqrt(var+eps)."""
                bnst = stpool.tile([P, 2, 6], F32, tag=f"bn{tag}", name="bnst")
                xg = src.rearrange("p (n f) -> p n f", f=384)
                for g in range(2):
                    nc.vector.bn_stats(bnst[:, g, :], xg[:, g, :])
                mv = stpool.tile([P, 2], F32, tag=f"mv{tag}", name="mv")
                nc.vector.bn_aggr(mv[:], bnst[:])
                rt = stpool.tile([P, 1], F32, tag=f"rt{tag}", name="rt")
                nc.scalar.activation(rt[:], mv[:, 1:2], AF.Abs_reciprocal_sqrt,
                                     bias=eps_b[:])
                return mv, rt

            with tc.tile_pool(name="fm", bufs=1) as fmp:
                z1T = fmp.tile([P, EC, L], MDT, tag="fm", name="z1T")
                with tc.tile_pool(name="wfcp", bufs=1) as wfcp:
                    wfc_sb = wfcp.tile([P, EC, 4 * E], MDT)
                    with (
                        tc.tile_pool(name="otp", bufs=1) as otp,
                        tc.tile_pool(name="vp", bufs=1) as vpool,
                        tc.tile_pool(name="wpp", bufs=1) as wpp,
                    ):
                        OT = otp.tile([P, EC, L], MDT)
                        v_aug = vpool.tile([P, LT, H, DA], MDT)
                        wproj_sb = wpp.tile([P, EC, E], MDT)

                        # ---- ph0+ph2 fused: per-tile LN1 -> z1T -> V ----
                        with (
                            tc.tile_pool(name="wvp", bufs=1) as wvp,
                            tc.tile_pool(name="zp", bufs=3) as zp,
                            tc.tile_pool(name="stp", bufs=4) as stp,
                            tc.tile_pool(name="ps0", bufs=4, space="PSUM") as ps0,
                            tc.tile_pool(name="ps2", bufs=4, space="PSUM") as ps2,
                        ):
                            wv_sb = wvp.tile([P, EC, E], MDT)
                            nc.vector.tensor_copy(
                                v_aug[:, :, :, D:DA],
                                ones_f32[:].rearrange("p (t h o) -> p t h o",
                                                      h=H, o=1))
                            # x tiles 0-1 already in flight (issued at pers
                            # setup); wv next (needed by V at ~7us), then the
                            # rest of x, then wproj.
                            nc.sync.dma_start(wv_sb[:], wvv[:])
                            for t in range(2, LT):
                                nc.sync.dma_start(xall[:, t, :], xv[:, t, :])
                            nc.sync.dma_start(wproj_sb[:], wprojv[:])
                            # LN1 stats run 3 tiles ahead so the ACT rsqrt is
                            # never queued behind transpose copies.
                            stats1 = {}
                            for t in range(3):
                                stats1[t] = ln_tile(xall[:, t, :], stp, "1")
                            for t in range(LT):
                                if t + 3 < LT:
                                    stats1[t + 3] = ln_tile(
                                        xall[:, t + 3, :], stp, "1")
                                mv, rt = stats1.pop(t)
                                zt = zp.tile([P, E], MDT, tag="z")
                                nc.vector.tensor_scalar(
                                    zt[:], xall[:, t, :], mv[:, 0:1], rt[:],
                                    OP.subtract, OP.mult,
                                )
                                transpose_block(z1T, zt, t, ps0)
                                # V matmuls for tile t
                                for (c0, cw) in ((0, 512), (512, 256)):
                                    pt2 = ps2.tile([P, 512], F32, tag="mm")
                                    for kc in range(EC):
                                        nc.tensor.matmul(
                                            pt2[:, :cw],
                                            z1T[:, kc, t * P:(t + 1) * P],
                                            wv_sb[:, kc, c0:c0 + cw],
                                            start=(kc == 0), stop=(kc == EC - 1),
                                        )
                                    h0 = c0 // D
                                    nh = cw // D
                                    dst = v_aug[:, t, h0:h0 + nh, 0:D]
                                    if gates["bv"]:
                                        nc.vector.tensor_tensor(
                                            dst,
                                            pt2[:, :cw].rearrange(
                                                "p (h d) -> p h d", d=D),
                                            bv_sb[:, c0:c0 + cw].rearrange(
                                                "p (h d) -> p h d", d=D),
                                            OP.add)
                                    else:
                                        nc.vector.tensor_copy(
                                            out=dst,
                                            in_=pt2[:, :cw].rearrange(
                                                "p (h d) -> p h d", d=D))

                        # ---- ph3: per head pair qk + attention ----
                        # Cycle emission: S^T(all kt) -> PV(q0) -> PV(q1) ->
                        # norms -> qk(c+1).  The qk stream at the cycle end is
                        # 5us of independent PE work during which the next
                        # cycle's psum->SBUF copies and this cycle's norm
                        # chains drain, so the PE re-enters S^T with all
                        # dependencies met.  qk borrows the [P, L] S^T psum
                        # tiles (both lc halves in one tile); PV has its own
                        # 4-bank pool so no intra-cycle WAR aliasing exists.
                        with (
                            tc.tile_pool(name="qkpp", bufs=3) as qkpp,
                            tc.tile_pool(name="wqks", bufs=4) as wqs,
                            tc.tile_pool(name="ptp", bufs=1) as ptp,
                            tc.tile_pool(name="recp", bufs=3) as recp,
                            tc.tile_pool(name="ps3", bufs=4, space="PSUM") as ps3,
                            tc.tile_pool(name="ps3s", bufs=2, space="PSUM") as ps3s,
                        ):
                            # preload the Exp table during qk(0) so the first
                            # real exp doesn't eat a 1.3us table load
                            nc.scalar.activation(tabs[:], tabs[:], AF.Exp)

                            def emit_qk(c):
                                qk_pair = qkpp.tile([P, 2, L], MDT, tag="qkpair",
                                                    name=f"qkp{c}")
                                for i, oc in enumerate((c, EC + c)):
                                    wt = wqs.tile([P, EC, P], MDT, tag="wqk",
                                                  name=f"wqk{c}_{i}")
                                    nc.sync.dma_start(
                                        wt[:], wqkv[:, :, oc * P:(oc + 1) * P])
                                    psums = [ps3.tile([P, 512], F32, tag="mm",
                                                      name=f"qkps{lc}")
                                             for lc in range(QC)]
                                    for kc in range(EC):
                                        for lc in range(QC):
                                            nc.tensor.matmul(
                                                psums[lc][:], wt[:, kc, :],
                                                z1T[:, kc, lc * 512:(lc + 1) * 512],
                                                start=(kc == 0),
                                                stop=(kc == EC - 1),
                                            )
                                    for lc in range(QC):
                                        dst = qk_pair[:, i, lc * 512:(lc + 1) * 512]
                                        if gates["bqk"]:
                                            nc.scalar.activation(
                                                dst, psums[lc][:], AF.Identity,
                                                bias=bqk_sb[:, oc:oc + 1])
                                        else:
                                            # keep these on DVE: a cast on
                                            # the ACT queue sits between exps
                                            # and delays the psum-ring WARs
                                            nc.vector.tensor_copy(
                                                out=dst, in_=psums[lc][:])
                                return qk_pair

                            def emit_st(qk_pair, PTs, kt):
                                # S^T for key tile kt: full row s0:L in one
                                # [P, L] psum tile so same-lhsT matmuls go
                                # back-to-back (pipelined drains), exp split
                                # per query half into PTs[qc][par].
                                s0 = kt * P
                                segs = ([(s0, 512), (512, L)] if s0 < 512
                                        else [(s0, L)])
                                psss = [ps3s.tile([P, L], F32, tag="st",
                                                  name=f"pss{par}")
                                        for par in range(2)]
                                # seg-major emission: the two parities of one
                                # segment are FIFO-adjacent, so their disjoint
                                # PE row groups run concurrently (row packing);
                                # par-major order lets par0's second segment
                                # (same rows as its first) block par1's start.
                                for (a, b) in segs:
                                    for par in range(2):
                                        rows = slice(par * D, par * D + D)
                                        nc.tensor.matmul(
                                            psss[par][:, a:b],
                                            qk_pair[rows, 1, s0:s0 + P],
                                            qk_pair[rows, 0, a:b],
                                            start=True, stop=True)
                                for par in range(2):
                                    pss = psss[par]
                                    if s0 < 512:
                                        nc.scalar.activation(
                                            PTs[0][par][:, kt, s0:512],
                                            pss[:, s0:512], AF.Exp)
                                        nc.scalar.activation(
                                            PTs[1][par][:, kt, 0:512],
                                            pss[:, 512:L], AF.Exp)
                                        nc.vector.tensor_tensor(
                                            PTs[0][par][:, kt, s0:s0 + P],
                                            PTs[0][par][:, kt, s0:s0 + P],
                                            mask_tri, OP.mult)
                                    else:
                                        nc.scalar.activation(
                                            PTs[1][par][:, kt, s0 - 512:512],
                                            pss[:, s0:L], AF.Exp)
                                        nc.vector.tensor_tensor(
                                            PTs[1][par][:, kt,
                                                        s0 - 512:s0 - 512 + P],
                                            PTs[1][par][:, kt,
                                                        s0 - 512:s0 - 512 + P],
                                            mask_tri, OP.mult)

                            def emit_pv_part(c, qc, PTx, psos, kts, first, last):
                                # P@V accumulation sub-group over key tiles
                                # `kts`; the group may be split around other
                                # matmuls (start only on `first`, stop on
                                # `last`).
                                q0 = qc * 512
                                for par in range(2):
                                    h = 2 * c + par
                                    pt_buf = PTx[par]
                                    pso = psos[par]
                                    for idx, j in enumerate(kts):
                                        a = max(j * P, q0)
                                        nc.tensor.matmul(
                                            pso[0:DA, a - q0:512],
                                            v_aug[:, j, h, :],
                                            pt_buf[:, j, a - q0:512],
                                            start=(first and idx == 0),
                                            stop=(last and idx == len(kts) - 1))

                            def emit_pv_norm(c, qc, psos):
                                # psum row 64 carries softmax row-sums (the
                                # ones column of v_aug).  Copy O^T and the
                                # sums row straight out of PSUM so the PV
                                # bank frees after the copies (the bank's
                                # next writer — the coming cycle's qk
                                # matmuls — was measured stalling 1.5us/pair
                                # waiting on the old copy->recip->broadcast->
                                # mult chain); the normalize then runs from
                                # SBUF off the bank-release path.  The
                                # reciprocal input must sit at partition 0
                                # (the custom-DVE seed NaNs on a base-64 row).
                                q0 = qc * 512
                                for par in range(2):
                                    pso = psos[par]
                                    o_rows = slice(par * D, par * D + D)
                                    osb = recp.tile([P, 512], F32, tag="ob")
                                    if qc == 0:
                                        # ACT's queue is exp-free right here
                                        nc.scalar.copy(out=osb[0:D, :],
                                                       in_=pso[0:D, :])
                                    else:
                                        nc.vector.tensor_copy(osb[0:D, :],
                                                              pso[0:D, :])
                                    srow = recp.tile([P, 512], F32, tag="sr")
                                    nc.vector.tensor_copy(srow[0:1, :],
                                                          pso[D:DA, :])
                                    rec = recp.tile([P, 512], F32, tag="rc")
                                    nc.vector.reciprocal_approx_fast(
                                        rec[0:1, :], srow[0:1, :])
                                    recb = recp.tile([P, 512], F32, tag="rb")
                                    nc.gpsimd.partition_broadcast(
                                        recb[0:D, :], rec[0:1, :])
                                    nc.vector.tensor_tensor(
                                        OT[o_rows, c, q0:q0 + 512],
                                        osb[0:D, :], recb[0:D, :], OP.mult,
                                    )

                            # Per-pair cycle, software-pipelined so the PE
                            # never waits on the exp/mask/copy chains:
                            #   S^T(kt0-3) -> qk(c+1) [5us of independent PE
                            #   work while exps+masks drain] -> PV(q0) ->
                            #   PV(q1) over kt0-3 -> S^T(kt4-7) -> PV(q1)
                            #   tail.  The PV(q1) accumulation group stays
                            #   open across the second S^T batch.
                            qkp_cur = emit_qk(0)
                            for c in range(EC):  # head pair (2c, 2c+1)
                                PTs = [[ptp.tile([P, LT, 512], MDT,
                                                 tag=f"pt{qc}{par}",
                                                 name=f"pt{qc}{par}")
                                        for par in range(2)]
                                       for qc in range(QC)]
                                for kt in range(4):
                                    emit_st(qkp_cur, PTs, kt)
                                qkp_next = (emit_qk(c + 1) if c + 1 < EC
                                            else None)
                                psos0 = [ps3.tile([P, 512], F32, tag="mm",
                                                  name=f"pv0{par}")
                                         for par in range(2)]
                                psos1 = [ps3.tile([P, 512], F32, tag="mm",
                                                  name=f"pv1{par}")
                                         for par in range(2)]
                                emit_pv_part(c, 0, PTs[0], psos0,
                                             range(4), True, True)
                                emit_pv_part(c, 1, PTs[1], psos1,
                                             range(4), True, False)
                                for kt in range(4, LT):
                                    emit_st(qkp_cur, PTs, kt)
                                emit_pv_norm(c, 0, psos0)
                                emit_pv_part(c, 1, PTs[1], psos1,
                                             range(4, LT), False, True)
                                emit_pv_norm(c, 1, psos1)
                                qkp_cur = qkp_next
                            # reload the rsqrt table while the PE finishes the
                            # last pair, so ph4's first LN2 chain doesn't wait
                            # the 1.3us load inside the phase-boundary gap
                            nc.scalar.activation(tabs[:], tabs[:],
                                                 AF.Abs_reciprocal_sqrt)

                        # ---- ph4: proj + residual + LN2 + transpose ----
                        z2T = fmp.tile([P, EC, L], MDT, tag="fm", name="z2T")
                        with (
                            tc.tile_pool(name="zp2", bufs=3) as zp2,
                            tc.tile_pool(name="stp2", bufs=4) as stp2,
                            tc.tile_pool(name="ps4", bufs=4, space="PSUM") as ps4,
                            tc.tile_pool(name="ps45", bufs=4, space="PSUM") as ps45,
                        ):
                            # prefetch whole wfc during ph4 (8 chunks so ph5's
                            # first oc doesn't wait on one giant DMA)
                            for j in range(8):
                                nc.sync.dma_start(
                                    wfc_sb[:, :, j * 384:(j + 1) * 384],
                                    wfcv[:, :, j * 384:(j + 1) * 384])
                            prev_z2t = None
                            for t in range(LT):
                                for (c0, cw) in ((0, 512), (512, 256)):
                                    pt = ps4.tile([P, 512], F32, tag="mm")
                                    for kc in range(EC):
                                        nc.tensor.matmul(
                                            pt[:, :cw],
                                            OT[:, kc, t * P:(t + 1) * P],
                                            wproj_sb[:, kc, c0:c0 + cw],
                                            start=(kc == 0), stop=(kc == EC - 1),
                                        )
                                    dst = x1all[:, t, c0:c0 + cw]
                                    if gates["bproj"]:
                                        nc.vector.tensor_tensor(
                                            dst, pt[:, :cw],
                                            bproj_sb[:, c0:c0 + cw], OP.add)
                                        nc.vector.tensor_tensor(
                                            dst, dst, xall[:, t, c0:c0 + cw],
                                            OP.add)
                                    else:
                                        nc.vector.tensor_tensor(
                                            dst, pt[:, :cw],
                                            xall[:, t, c0:c0 + cw], OP.add)
                                mv2, rt2 = ln_tile(x1all[:, t, :], stp2, "2")
                                z2t = zp2.tile([P, E], MDT, tag="z2")
                                nc.vector.tensor_scalar(
                                    z2t[:], x1all[:, t, :], mv2[:, 0:1], rt2[:],
                                    OP.subtract, OP.mult,
                                )
                                # transposes lag one tile so the PE never
                                # waits on the DVE stats/apply chain
                                if prev_z2t is not None:
                                    transpose_block(z2T, prev_z2t, t - 1, ps45)
                                prev_z2t = z2t
                            transpose_block(z2T, prev_z2t, LT - 1, ps45)
                            # preload the exp table for ph5's selu during the
                            # last transposes / first fc matmuls
                            nc.scalar.activation(tabs[:], tabs[:], AF.Exp)

                    # ---- ph5: fc + selu -> hT ----
                    with (
                        tc.tile_pool(name="htp", bufs=1) as htp,
                        tc.tile_pool(name="wop", bufs=1) as wop,
                    ):
                        hT = htp.tile([P, KC2, L], MDT)
                        wo_a = wop.tile([P, KC2, 512], MDT)
                        nc.sync.dma_start(wo_a[:], woutv[:, :, 0:512])
                        wo_b = wop.tile([P, KC2, 256], MDT)
                        nc.sync.dma_start(wo_b[:], woutv[:, :, 512:768])
                        with (
                            tc.tile_pool(name="selu", bufs=2) as slp,
                            # one psum ring spans ph5 AND ph6 so the phase
                            # boundary has no pool-reuse WAR gap
                            tc.tile_pool(name="ps5", bufs=4, space="PSUM") as ps5,
                        ):
                            for oc in range(KC2):
                                for lc in range(QC):
                                    pt = ps5.tile([P, 512], F32, tag="mm")
                                    for kc in range(EC):
                                        nc.tensor.matmul(
                                            pt[:],
                                            wfc_sb[:, kc, oc * P:(oc + 1) * P],
                                            z2T[:, kc, lc * 512:(lc + 1) * 512],
                                            start=(kc == 0), stop=(kc == EC - 1),
                                        )
                                    pe_t = slp.tile([P, 512], F32, tag="pe")
                                    bias = (bfce_sb[:, oc:oc + 1] if gates["bfc"]
                                            else lnla_b[:])
                                    nc.scalar.activation(pe_t[:], pt[:], AF.Exp,
                                                         bias=bias,
                                                         scale=1.0 / SELU_LAMBDA)
                                    a_t = slp.tile([P, 512], F32, tag="at")
                                    nc.vector.tensor_scalar(
                                        a_t[:], pe_t[:], SELU_LA, SELU_LA,
                                        OP.min, OP.subtract)
                                    dst = hT[:, oc, lc * 512:(lc + 1) * 512]
                                    if gates["bfc"]:
                                        rl = slp.tile([P, 512], F32, tag="rl")
                                        nc.vector.tensor_scalar(
                                            rl[:], pt[:], bfcl_sb[:, oc:oc + 1],
                                            0.0, OP.add, OP.max)
                                        nc.vector.tensor_tensor(dst, rl[:],
                                                                a_t[:], OP.add)
                                    else:
                                        nc.vector.scalar_tensor_tensor(
                                            dst, pt[:], 0.0, a_t[:],
                                            OP.max, OP.add)

                            # ---- ph6: out = h @ wout + x1 (two passes) ----
                            with tc.tile_pool(name="osp", bufs=3) as osp:
                                ps6 = ps5
                            for t in range(LT):
                                pt = ps6.tile([P, 512], F32, tag="mm")
                                for kc in range(KC2):
                                    nc.tensor.matmul(
                                        pt[:], hT[:, kc, t * P:(t + 1) * P],
                                        wo_a[:, kc, :],
                                        start=(kc == 0), stop=(kc == KC2 - 1),
                                    )
                                ot = osp.tile([P, 512], F32, tag="ot")
                                if gates["bout"]:
                                    nc.vector.tensor_tensor(
                                        ot[:], pt[:], bout_sb[:, 0:512], OP.add)
                                    nc.vector.tensor_tensor(
                                        ot[:], ot[:], x1all[:, t, 0:512], OP.add)
                                else:
                                    nc.vector.tensor_tensor(
                                        ot[:], pt[:], x1all[:, t, 0:512], OP.add)
                                nc.sync.dma_start(outv[:, t, 0:512], ot[:])

                            for t in range(LT):
                                pt = ps6.tile([P, 512], F32, tag="mm")
                                for kc in range(KC2):
                                    nc.tensor.matmul(
                                        pt[:, :256], hT[:, kc, t * P:(t + 1) * P],
                                        wo_b[:, kc, :],
                                        start=(kc == 0), stop=(kc == KC2 - 1),
                                    )
                                ot = osp.tile([P, 512], F32, tag="ot")
                                if gates["bout"]:
                                    nc.vector.tensor_tensor(
                                        ot[:, :256], pt[:, :256],
                                        bout_sb[:, 512:768], OP.add)
                                    nc.vector.tensor_tensor(
                                        ot[:, :256], ot[:, :256],
                                        x1all[:, t, 512:768], OP.add)
                                else:
                                    nc.vector.tensor_tensor(
                                        ot[:, :256], pt[:, :256],
                                        x1all[:, t, 512:768], OP.add)
                                nc.sync.dma_start(outv[:, t, 512:768],
                                                  ot[:, :256])

    nc.finalize()
    return nc


def kernel(**inputs):
    global _last_results

    mm_dt_name = os.environ.get("KERNEL_MM_DT", "bf16")

    def arr(name):
        return np.ascontiguousarray(np.asarray(inputs[name], dtype=np.float32))

    x = arr("x")                       # [8, 1024, 768]
    g1 = arr("ln1_scale")
    b1 = arr("ln1_bias")
    w_qkv = arr("w_qkv")               # [768, 2304]
    b_qkv = arr("b_qkv")
    w_proj = arr("w_proj")
    b_proj = arr("b_proj")
    g2 = arr("ln2_scale")
    b2 = arr("ln2_bias")
    w_fc = arr("w_fc")
    b_fc = arr("b_fc")
    w_out = arr("w_out")
    b_out = arr("b_out")

    qscale = np.float32(1.0 / np.sqrt(D))

    w3 = w_qkv.reshape(E, H, 3, D)
    qw = (w3[:, :, 0, :].reshape(E, E) * qscale)
    kw = w3[:, :, 1, :].reshape(E, E)
    vw = w3[:, :, 2, :].reshape(E, E)
    wqk = np.ascontiguousarray(
        np.concatenate([qw, kw], axis=1) * g1[:, None]).astype(np.float32)
    wv = np.ascontiguousarray(vw * g1[:, None]).astype(np.float32)

    bq3 = (b1 @ w_qkv + b_qkv).reshape(H, 3, D)
    bqk = np.concatenate(
        [bq3[:, 0, :].reshape(E) * qscale, bq3[:, 1, :].reshape(E)]).astype(np.float32)
    bv = np.ascontiguousarray(bq3[:, 2, :].reshape(E)).astype(np.float32)

    wfc_p = np.ascontiguousarray(
        w_fc * g2[:, None] * np.float32(SELU_LAMBDA)).astype(np.float32)
    bfc_eff = (b2 @ w_fc + b_fc).astype(np.float32)
    bfce = (bfc_eff + np.float32(np.log(SELU_LA))).astype(np.float32)
    bfcl = (bfc_eff * np.float32(SELU_LAMBDA)).astype(np.float32)

    gates = {
        "bqk": bool(np.any(bqk != 0)),
        "bv": bool(np.any(bv != 0)),
        "bproj": bool(np.any(b_proj != 0)),
        "bfc": bool(np.any(bfc_eff != 0)),
        "bout": bool(np.any(b_out != 0)),
    }

    key = (tuple(sorted(gates.items())), mm_dt_name)
    if key not in _build_cache:
        _build_cache[key] = _build(gates, mm_dt_name)
    nc = _build_cache[key]

    wdt = np.float32 if mm_dt_name == "f32r" else ml_dtypes.bfloat16

    def wcast(a):
        return np.ascontiguousarray(a.astype(wdt))

    base = {
        "wqk": wcast(wqk), "wv": wcast(wv),
        "wproj": wcast(w_proj),
        "wfc": wcast(wfc_p),
        "wout": wcast(w_out),
    }
    if gates["bqk"]:
        base["bqk"] = bqk
    if gates["bv"]:
        base["bv"] = bv
    if gates["bproj"]:
        base["bproj"] = np.ascontiguousarray(b_proj)
    if gates["bfc"]:
        base["bfce"] = bfce
        base["bfcl"] = bfcl
    if gates["bout"]:
        base["bout"] = np.ascontiguousarray(b_out)

    in_maps = [dict(base, x=np.ascontiguousarray(x[c])) for c in range(NCORES)]
    res = bass_utils.run_bass_kernel_spmd(nc, in_maps, core_ids=list(range(NCORES)))
    _last_results = res
    out = np.stack([res.results[c]["out"] for c in range(NCORES)], axis=0)
    return out.astype(np.float32)
